# revision 21
# baseline (speedup 1.0000x reference)
"""Trainium2 Bass kernel for Linformer-style sparse attention.

Problem shapes (hardcoded): B=4, S=4096, D=1024, H=16, HD=64, LK=256.

Sharding (8 cores): core c -> (batch b = c//2, sequence half = c%2).
Each core:
  - computes Q/K/V for its 2048 rows (all heads),
  - computes partial [Kp^T; Vp^T] = (K|V)^T @ E^T over its rows,
  - pair AllReduce ([0,1],[2,3],[4,5],[6,7]) completes Kp/Vp (2x 512 KiB bf16),
  - attention (softmax over LK=256) + output projection for its own rows,
  - writes its [2048, 1024] slice of the output directly (no final collective).

Key layout decision: X and E are passed to the device ALREADY TRANSPOSED
(prepared on the host as part of input sharding): XT = X_slice^T [D, SL] and
EP = head-pair-packed E^T [H/2, SL, 2*LK].  On-device they are cast-loaded
f32->bf16 with fully contiguous >=2KB reads straight into matmul-ready SBUF
layouts.  This removes the on-device XBAR transposes of X and E (the previous
bottleneck: tens of thousands of 256-byte transpose packets that saturated
the DMA engines for ~400us and starved the PE).

Other scheduling choices:
  - XT is cast in 4 s-chunks so the first K matmul starts ~20us in.
  - K loop runs before the V loop so only Wk gates compute start.
  - Q^T is computed in 32 (mc, sn) pieces woven two-per-head between partial
    heads; E pairs stream through a 4-deep pool during the K/V phase.
  - The pair AllReduce is split in half (heads 0-7 fire mid-weave, 8-15 at
    the end); attention runs j-outer so the j<4 pair-groups (AR0 results)
    cover AR1's latency; Vp is un-transposed by tiny per-head XBAR DMAs.
  - kp writes and cc readbacks ride the HWDGE (sync) queue; Wo's cast is
    issued after the weave so nothing blocks behind the collectives.
"""

import sys

sys.path.insert(0, "/opt/trn_rl_repo")

from contextlib import ExitStack

import numpy as np

from concourse import bacc, bass_utils, mybir, tile

B, S, D = 4, 4096, 1024
H, HD, LK = 16, 64, 256
SL = S // 2            # local sequence rows per core
P = 128
NSC = SL // P          # 16 s-chunks of 128
NDC = D // P           # 8 d-chunks of 128
NSN = SL // 512        # 4 s-chunks of 512
f32 = mybir.dt.float32
bf16 = mybir.dt.bfloat16
PAIRS = [[0, 1], [2, 3], [4, 5], [6, 7]]


def _build(include_biases: bool, debug: bool = False):
    nc = bacc.Bacc("TRN2", target_bir_lowering=False, num_devices=8)

    XT_e = nc.declare_dram_parameter("XT", [D, SL], f32, isOutput=False)
    mask_e = nc.declare_dram_parameter("mask", [SL], f32, isOutput=False)
    Wq_e = nc.declare_dram_parameter("Wq", [D, D], f32, isOutput=False)
    bq_e = nc.declare_dram_parameter("bq", [D], f32, isOutput=False)
    Wk_e = nc.declare_dram_parameter("Wk", [D, D], f32, isOutput=False)
    bk_e = nc.declare_dram_parameter("bk", [D], f32, isOutput=False)
    Wv_e = nc.declare_dram_parameter("Wv", [D, D], f32, isOutput=False)
    bv_e = nc.declare_dram_parameter("bv", [D], f32, isOutput=False)
    EP_e = nc.declare_dram_parameter("EP", [H // 2, SL, 2 * LK], f32, isOutput=False)
    Wo_e = nc.declare_dram_parameter("Wo", [D, D], f32, isOutput=False)
    bo_e = nc.declare_dram_parameter("bo", [D], f32, isOutput=False)
    out_e = nc.declare_dram_parameter("out", [SL, D], f32, isOutput=True)

    # AllReduce bounce (bf16): per head [KpT ; VpT] stacked [128, 256] flat
    cc_in = nc.dram_tensor("cc_in", [H, P * LK], bf16, kind="Internal")
    cc_out = nc.dram_tensor("cc_out", [H, P * LK], bf16, kind="Internal")

    with tile.TileContext(nc) as tc:
        ctx = ExitStack()
        with ctx:
            const_pool = ctx.enter_context(tc.tile_pool(name="consts", bufs=1))

            # ---------------- constants ----------------
            m_sb = const_pool.tile([P, NSC], f32, name="m_sb")
            nc.sync.dma_start(m_sb[:], mask_e.ap().rearrange("(o p) -> p o", p=P))
            bq_sb = const_pool.tile([P, NDC], f32, name="bq_sb")
            nc.sync.dma_start(bq_sb[:], bq_e.ap().rearrange("(o p) -> p o", p=P))
            if include_biases:
                bkv_bc = const_pool.tile([P, 2, D], f32, name="bkv_bc")
                nc.sync.dma_start(bkv_bc[:, 0, :], bk_e.ap()[None, :].to_broadcast((P, D)))
                nc.sync.dma_start(bkv_bc[:, 1, :], bv_e.ap()[None, :].to_broadcast((P, D)))
            ones_sb = const_pool.tile([P, HD], bf16, name="ones_sb")
            nc.vector.memset(ones_sb[:], 1.0)
            bo_bc = const_pool.tile([P, D], bf16, name="bo_bc")

            # ---------------- left-stack long-lived tiles ----------------
            # xT layout: [d_in(P), dc(8), s(2048)]  (d = dc*128 + d_in)
            xT, free_xT = tc.tile([P, NDC, SL], bf16, name="xT")

            # ---------------- right-stack: freed in reverse order ----------------
            eT_scope = ExitStack()
            eTT_pool = eT_scope.enter_context(
                tc.tile_pool(name="eTT", bufs=3, side="right")
            )
            part_pool = eT_scope.enter_context(
                tc.tile_pool(name="part", bufs=3, side="right")
            )
            kv, free_kv = tc.tile([P, NSC, H, 2, HD], bf16, name="kv", side="right")
            wv_bf, free_wv = tc.tile([P, NDC, D], bf16, name="wv_bf", side="right")
            wk_bf, free_wk = tc.tile([P, NDC, D], bf16, name="wk_bf", side="right")

            # ---------------- cast loads (issue order = SWDGE drain order) --------
            def load_x_chunk(sn):
                nc.gpsimd.dma_start(
                    out=xT[:, :, sn * 512 : (sn + 1) * 512],
                    in_=XT_e[:, sn * 512 : (sn + 1) * 512].rearrange(
                        "(o p) n -> p o n", p=P
                    ),
                )

            wq_bf, free_wq = tc.tile([P, NDC, D], bf16, name="wq_bf")

            def load_w_half(w_bf, W_e, half):
                nc.gpsimd.dma_start(
                    out=w_bf[:, :, half * 512 : (half + 1) * 512],
                    in_=W_e[:, half * 512 : (half + 1) * 512].rearrange(
                        "(o p) n -> p o n", p=P
                    ),
                )

            load_x_chunk(0)
            load_w_half(wk_bf, Wk_e, 0)
            load_x_chunk(1)
            load_w_half(wk_bf, Wk_e, 1)
            load_x_chunk(2)
            load_x_chunk(3)
            nc.gpsimd.dma_start(
                out=wv_bf[:], in_=Wv_e.ap().rearrange("(o p) n -> p o n", p=P)
            )
            load_w_half(wq_bf, Wq_e, 0)

            # E pair-tiles: eTT[s_in, so, (jh, k)] bf16, contiguous 2KB reads
            eTT_tiles = {}

            def stage_eTT(pr):
                eTT = eTT_pool.tile([P, NSC, 2 * LK], bf16, name="eTT")
                nc.gpsimd.dma_start(
                    out=eTT[:],
                    in_=EP_e[pr].rearrange("(o p) n -> p o n", p=P),
                )
                eTT_tiles[pr] = eTT

            stage_eTT(0)
            stage_eTT(1)
            load_w_half(wq_bf, Wq_e, 1)
            stage_eTT(2)

            with (
                tc.tile_pool(name="ps_kvq", bufs=4, space="PSUM") as ps_kvq,
                tc.tile_pool(name="ps_part", bufs=2, space="PSUM") as ps_part,
            ):
                # ---------------- K then V, natural [s, dh] (masked, bf16) ----------
                for t, w_bf in ((0, wk_bf), (1, wv_bf)):
                    for half in range(2):
                        for sc in range(NSC):
                            ps = ps_kvq.tile([P, 512], f32, name="ps_kv", tag="mm512")
                            for dc in range(NDC):
                                nc.tensor.matmul(
                                    ps[:],
                                    xT[:, dc, sc * P : (sc + 1) * P],
                                    w_bf[:, dc, half * 512 : (half + 1) * 512],
                                    start=(dc == 0),
                                    stop=(dc == NDC - 1),
                                )
                            if include_biases:
                                nc.vector.tensor_tensor(
                                    out=ps[:],
                                    in0=ps[:],
                                    in1=bkv_bc[:, t, half * 512 : (half + 1) * 512],
                                    op=mybir.AluOpType.add,
                                )
                            nc.vector.tensor_scalar(
                                out=kv[:, sc, half * 8 : (half + 1) * 8, t, :],
                                in0=ps[:],
                                scalar1=m_sb[:, sc : sc + 1],
                                scalar2=None,
                                op0=mybir.AluOpType.mult,
                            )
                    if t == 0:
                        free_wk()
                free_wv()

                # Allocated only now: reuses the space wk/wv vacated.
                qT_lo, free_qT_lo = tc.tile([P, 4, SL], bf16, name="qT_lo")
                qT_hi, free_qT_hi = tc.tile([P, 4, SL], bf16, name="qT_hi")

                def q_piece(piece):
                    mc, sn = piece // NSN, piece % NSN
                    dst, dj = (qT_lo, 0) if mc < 4 else (qT_hi, 4)
                    ps = ps_kvq.tile([P, 512], f32, name="psq", tag="mm512")
                    for dc in range(NDC):
                        nc.tensor.matmul(
                            ps[:],
                            wq_bf[:, dc, mc * P : (mc + 1) * P],
                            xT[:, dc, sn * 512 : (sn + 1) * 512],
                            start=(dc == 0),
                            stop=(dc == NDC - 1),
                        )
                    nc.vector.tensor_scalar(
                        out=dst[:, mc - dj, sn * 512 : (sn + 1) * 512],
                        in0=ps[:],
                        scalar1=bq_sb[:, mc : mc + 1],
                        scalar2=None,
                        op0=mybir.AluOpType.add,
                    )

                # ---------------- partials per head, Q pieces woven in --------------
                for h in range(H):
                    q_piece(2 * h)
                    q_piece(2 * h + 1)
                    eTT = eTT_tiles[h // 2]
                    jh = h % 2
                    kp_ps = ps_part.tile([P, LK], f32, name="kp_ps")
                    for so in range(NSC):
                        nc.tensor.matmul(
                            kp_ps[:],
                            kv[:, so, h, :, :],
                            eTT[:, so, jh * LK : (jh + 1) * LK],
                            start=(so == 0),
                            stop=(so == NSC - 1),
                        )
                    kp_sb = part_pool.tile([P, LK], bf16, name="kp_sb")
                    nc.vector.tensor_copy(kp_sb[:], kp_ps[:])
                    nc.sync.dma_start(
                        out=cc_in[h].rearrange("(a b) -> a b", a=P),
                        in_=kp_sb[:],
                    )
                    if h % 2 == 1:
                        pr = h // 2
                        eTT_tiles.pop(pr)
                        if pr + 3 < H // 2:
                            stage_eTT(pr + 3)
                    if h == 5:
                        # heads 0-5 AllReduce fires mid-weave; the rest of the
                        # weave covers its latency so attention can start at
                        # the weave's end.
                        nc.gpsimd.collective_compute(
                            "AllReduce",
                            mybir.AluOpType.add,
                            replica_groups=PAIRS,
                            ins=[cc_in[0:6].opt()],
                            outs=[cc_out[0:6].opt()],
                        )

                nc.gpsimd.collective_compute(
                    "AllReduce",
                    mybir.AluOpType.add,
                    replica_groups=PAIRS,
                    ins=[cc_in[6:16].opt()],
                    outs=[cc_out[6:16].opt()],
                )

            free_kv()
            eT_scope.close()

            # ---------------- Wo (needed only for the projection tail) ------------
            wo_bf, free_wo = tc.tile([P, NDC, D], bf16, name="wo_bf")
            nc.gpsimd.dma_start(
                out=wo_bf[:], in_=Wo_e.ap().rearrange("(o p) n -> p o n", p=P)
            )
            nc.gpsimd.dma_start(out=bo_bc[:], in_=bo_e.ap()[None, :].to_broadcast((P, D)))

            # ---------------- read back reduced Kp^T / Vp ----------------
            kpT, free_kpT = tc.tile([P, H // 2, LK], bf16, name="kpT")
            vp_sb2, free_vp = tc.tile([P, H, 2, HD], bf16, name="vp_sb2")

            def read_back(h):
                par = (h % 2) * 64
                nc.sync.dma_start(
                    out=kpT[par : par + 64, h // 2, :],
                    in_=cc_out[h, 0 : 64 * LK].rearrange("(a b) -> a b", a=64),
                )
                # Vp^T [hd, k] in DRAM -> XBAR transpose -> vp [k_in, kc, hd]
                nc.sync.dma_start_transpose(
                    vp_sb2[:, h, :, :],
                    cc_out[h, 64 * LK :].rearrange("(a b) -> a b", a=64),
                )

            # heads 0-5 gate only on the first AllReduce; heads 6-15 are read
            # back after two attention pair-groups so the PE covers AR1 latency.
            for h in range(6):
                read_back(h)

            # ---------------- attention (j outer) + output projection tail ---------
            xoT, free_xoT = tc.tile([P, NDC, SL], bf16, name="xoT")
            with (
                tc.tile_pool(name="at_pool", bufs=3) as at_pool,
                tc.tile_pool(name="rbc_pool", bufs=2) as rbc_pool,
                tc.tile_pool(name="osb_pool", bufs=3) as osb_pool,
                tc.tile_pool(name="ps_dot", bufs=2, space="PSUM") as ps_dot,
                tc.tile_pool(name="ps_xoden", bufs=4, space="PSUM") as ps_xoden,
                tc.tile_pool(name="ps_out", bufs=2, space="PSUM") as ps_out,
            ):
                if True:
                    def attn_pair(sn, j):
                        # heads (2j, 2j+1): even on partitions 0-63, odd on 64-127
                        ssl = slice(sn * 512, (sn + 1) * 512)
                        ats = []
                        for par in (0, 64):
                            at = at_pool.tile([P, 2, 512], bf16, name="at")
                            for kc in range(2):
                                dps = ps_dot.tile([P, 512], f32, name="dps")
                                qsrc = qT_lo if j < 4 else qT_hi
                                nc.tensor.matmul(
                                    dps[:],
                                    kpT[par : par + 64, j, kc * P : (kc + 1) * P],
                                    qsrc[par : par + 64, j % 4, ssl],
                                    start=True,
                                    stop=True,
                                )
                                nc.scalar.activation(
                                    out=at[:, kc, :],
                                    in_=dps[:],
                                    func=mybir.ActivationFunctionType.Exp,
                                    scale=0.125,
                                )
                            ats.append(at)
                        xo_ps = ps_xoden.tile([P, 512], f32, name="xo_ps", tag="xoden")
                        den_ps = ps_xoden.tile([P, 512], f32, name="den_ps", tag="xoden")
                        for kc in range(2):
                            for pi, par in ((0, 0), (1, 64)):
                                h = 2 * j + pi
                                nc.tensor.matmul(
                                    xo_ps[par : par + 64, :],
                                    vp_sb2[:, h, kc, :],
                                    ats[pi][:, kc, :],
                                    start=(kc == 0),
                                    stop=(kc == 1),
                                    skip_group_check=True,
                                )
                                nc.tensor.matmul(
                                    den_ps[par : par + 64, :],
                                    ones_sb[:],
                                    ats[pi][:, kc, :],
                                    start=(kc == 0),
                                    stop=(kc == 1),
                                    skip_group_check=True,
                                )
                        rbc = rbc_pool.tile([P, 512], f32, name="rbc")
                        nc.vector.reciprocal_approx_fast(out=rbc[:], in_=den_ps[:])
                        nc.vector.tensor_tensor(
                            out=xoT[:, j, ssl],
                            in0=xo_ps[:],
                            in1=rbc[:],
                            op=mybir.AluOpType.mult,
                        )

                    def proj_sn(sn):
                        for si in range(4):
                            sc = sn * 4 + si
                            for half in range(2):
                                ps = ps_out.tile([P, 512], f32, name="ps_o")
                                for c in range(NDC):
                                    nc.tensor.matmul(
                                        ps[:],
                                        xoT[:, c, sc * P : (sc + 1) * P],
                                        wo_bf[:, c, half * 512 : (half + 1) * 512],
                                        start=(c == 0),
                                        stop=(c == NDC - 1),
                                    )
                                osb = osb_pool.tile([P, 512], f32, name="osb")
                                nc.vector.tensor_tensor(
                                    out=osb[:],
                                    in0=ps[:],
                                    in1=bo_bc[:, half * 512 : (half + 1) * 512],
                                    op=mybir.AluOpType.add,
                                )
                                nc.sync.dma_start(
                                    out=out_e[sc * P : (sc + 1) * P, half * 512 : (half + 1) * 512],
                                    in_=osb[:],
                                )

                    # j-major while AR1 is in flight (j<3 only needs AR0)...
                    for j in range(3):
                        for sn in range(NSN):
                            attn_pair(sn, j)
                        if j == 1:
                            # AR1 has had two pair-groups (~20us) of PE cover
                            for h in range(6, H):
                                read_back(h)
                    # ...then sn-major so each sn-group's projection overlaps
                    # the next group's attention.
                    for sn in range(NSN):
                        for j in range(3, H // 2):
                            attn_pair(sn, j)
                        proj_sn(sn)
            free_xoT()
            free_vp()
            free_kpT()
            free_wo()
            free_qT_hi()
            free_qT_lo()
            free_wq()
            free_xT()

    nc.compile()
    return nc


_cache = {}


def _get_nc(include_biases: bool):
    if include_biases not in _cache:
        _cache[include_biases] = _build(include_biases)
    return _cache[include_biases]


def make_in_maps(inputs):
    X = np.asarray(inputs["X"], np.float32)
    mask = np.asarray(inputs["mask"], np.float32)
    E = np.asarray(inputs["E"], np.float32)
    Ws = {k: np.asarray(inputs[k], np.float32) for k in ("Wq", "Wk", "Wv", "Wo")}
    bs = {k: np.asarray(inputs[k], np.float32) for k in ("bq", "bk", "bv", "bo")}

    in_maps = []
    for c in range(8):
        b, half = c // 2, c % 2
        sl = slice(half * SL, (half + 1) * SL)
        # host-side layout prep: X^T and head-pair-packed E^T
        XT = np.ascontiguousarray(X[b, sl, :].T)                    # [D, SL]
        Esl = E[:, :, sl]                                           # [H, LK, SL]
        EP = np.ascontiguousarray(
            Esl.transpose(2, 0, 1).reshape(SL, H // 2, 2 * LK).transpose(1, 0, 2)
        )                                                           # [H/2, SL, 2*LK]
        in_maps.append(
            {
                "XT": XT,
                "mask": np.ascontiguousarray(mask[b, sl]),
                "Wq": Ws["Wq"], "bq": bs["bq"],
                "Wk": Ws["Wk"], "bk": bs["bk"],
                "Wv": Ws["Wv"], "bv": bs["bv"],
                "EP": EP,
                "Wo": Ws["Wo"], "bo": bs["bo"],
            }
        )
    include_biases = bool(np.any(bs["bk"]) or np.any(bs["bv"]))
    return in_maps, include_biases


def kernel(**inputs) -> np.ndarray:
    in_maps, include_biases = make_in_maps(inputs)
    nc = _get_nc(include_biases)
    res = bass_utils.run_bass_kernel_spmd(nc, in_maps, core_ids=list(range(8)))
    out = np.empty((B, S, D), np.float32)
    for c in range(8):
        b, half = c // 2, c % 2
        out[b, half * SL : (half + 1) * SL, :] = res.results[c]["out"]
    return out


# revision 22
# speedup vs baseline: 1.1116x; 1.1116x over previous
"""Trainium2 Bass kernel for Linformer-style sparse attention.

Problem shapes (hardcoded): B=4, S=4096, D=1024, H=16, HD=64, LK=256.

Sharding (8 cores): core c -> (batch b = c//2, sequence half = c%2).
Each core:
  - computes Q/K/V for its 2048 rows (all heads),
  - computes partial [Kp^T; Vp^T] = (K|V)^T @ E^T over its rows,
  - pair AllReduce ([0,1],[2,3],[4,5],[6,7]) completes Kp/Vp (2x 512 KiB bf16),
  - attention (softmax over LK=256) + output projection for its own rows,
  - writes its [2048, 1024] slice of the output directly (no final collective).

Key layout decision: X and E are passed to the device ALREADY TRANSPOSED
(prepared on the host as part of input sharding): XT = X_slice^T [D, SL] and
EP = head-pair-packed E^T [H/2, SL, 2*LK].  On-device they are cast-loaded
f32->bf16 with fully contiguous >=2KB reads straight into matmul-ready SBUF
layouts.  This removes the on-device XBAR transposes of X and E (the previous
bottleneck: tens of thousands of 256-byte transpose packets that saturated
the DMA engines for ~400us and starved the PE).

Other scheduling choices:
  - XT is cast in 4 s-chunks so the first K matmul starts ~20us in.
  - K loop runs before the V loop so only Wk gates compute start.
  - Q^T is computed in 32 (mc, sn) pieces woven two-per-head between partial
    heads; E pairs stream through a 4-deep pool during the K/V phase.
  - The pair AllReduce is split in half (heads 0-7 fire mid-weave, 8-15 at
    the end); attention runs j-outer so the j<4 pair-groups (AR0 results)
    cover AR1's latency; Vp is un-transposed by tiny per-head XBAR DMAs.
  - kp writes and cc readbacks ride the HWDGE (sync) queue; Wo's cast is
    issued after the weave so nothing blocks behind the collectives.
"""

import sys

sys.path.insert(0, "/opt/trn_rl_repo")

from contextlib import ExitStack

import numpy as np

from concourse import bacc, bass_utils, mybir, tile

B, S, D = 4, 4096, 1024
H, HD, LK = 16, 64, 256
SL = S // 2            # local sequence rows per core
P = 128
NSC = SL // P          # 16 s-chunks of 128
NDC = D // P           # 8 d-chunks of 128
NSN = SL // 512        # 4 s-chunks of 512
f32 = mybir.dt.float32
bf16 = mybir.dt.bfloat16
PAIRS = [[0, 1], [2, 3], [4, 5], [6, 7]]


def _build(include_biases: bool, debug: bool = False):
    nc = bacc.Bacc("TRN2", target_bir_lowering=False, num_devices=8)

    XT_e = nc.declare_dram_parameter("XT", [D, SL], f32, isOutput=False)
    mask_e = nc.declare_dram_parameter("mask", [SL], f32, isOutput=False)
    Wq_e = nc.declare_dram_parameter("Wq", [D, D], f32, isOutput=False)
    bq_e = nc.declare_dram_parameter("bq", [D], f32, isOutput=False)
    Wk_e = nc.declare_dram_parameter("Wk", [D, D], f32, isOutput=False)
    bk_e = nc.declare_dram_parameter("bk", [D], f32, isOutput=False)
    Wv_e = nc.declare_dram_parameter("Wv", [D, D], f32, isOutput=False)
    bv_e = nc.declare_dram_parameter("bv", [D], f32, isOutput=False)
    EP_e = nc.declare_dram_parameter("EP", [H // 2, SL, 2 * LK], f32, isOutput=False)
    Wo_e = nc.declare_dram_parameter("Wo", [D, D], f32, isOutput=False)
    bo_e = nc.declare_dram_parameter("bo", [D], f32, isOutput=False)
    out_e = nc.declare_dram_parameter("out", [SL, D], f32, isOutput=True)

    # AllReduce bounce (bf16): per head [KpT ; VpT] stacked [128, 256] flat
    cc_in = nc.dram_tensor("cc_in", [H, P * LK], bf16, kind="Internal")
    cc_out = nc.dram_tensor("cc_out", [H, P * LK], bf16, kind="Internal")

    with tile.TileContext(nc) as tc:
        ctx = ExitStack()
        with ctx:
            const_pool = ctx.enter_context(tc.tile_pool(name="consts", bufs=1))

            # ---------------- constants ----------------
            m_sb = const_pool.tile([P, NSC], f32, name="m_sb")
            nc.sync.dma_start(m_sb[:], mask_e.ap().rearrange("(o p) -> p o", p=P))
            bq_sb = const_pool.tile([P, NDC], f32, name="bq_sb")
            nc.sync.dma_start(bq_sb[:], bq_e.ap().rearrange("(o p) -> p o", p=P))
            if include_biases:
                bkv_bc = const_pool.tile([P, 2, D], f32, name="bkv_bc")
                nc.sync.dma_start(bkv_bc[:, 0, :], bk_e.ap()[None, :].to_broadcast((P, D)))
                nc.sync.dma_start(bkv_bc[:, 1, :], bv_e.ap()[None, :].to_broadcast((P, D)))
            ones_sb = const_pool.tile([P, HD], bf16, name="ones_sb")
            nc.vector.memset(ones_sb[:], 1.0)
            bo_bc = const_pool.tile([P, D], bf16, name="bo_bc")

            # ---------------- left-stack long-lived tiles ----------------
            # xT layout: [d_in(P), dc(8), s(2048)]  (d = dc*128 + d_in)
            xT, free_xT = tc.tile([P, NDC, SL], bf16, name="xT")

            # ---------------- right-stack: freed in reverse order ----------------
            eT_scope = ExitStack()
            eTT_pool = eT_scope.enter_context(
                tc.tile_pool(name="eTT", bufs=3, side="right")
            )
            part_pool = eT_scope.enter_context(
                tc.tile_pool(name="part", bufs=3, side="right")
            )
            kv, free_kv = tc.tile([P, NSC, H, 2, HD], bf16, name="kv", side="right")
            wv_bf, free_wv = tc.tile([P, NDC, D], bf16, name="wv_bf", side="right")
            wk_bf, free_wk = tc.tile([P, NDC, D], bf16, name="wk_bf", side="right")

            # ---------------- cast loads (issue order = SWDGE drain order) --------
            def load_x_chunk(sn):
                nc.gpsimd.dma_start(
                    out=xT[:, :, sn * 512 : (sn + 1) * 512],
                    in_=XT_e[:, sn * 512 : (sn + 1) * 512].rearrange(
                        "(o p) n -> p o n", p=P
                    ),
                )

            wq_bf, free_wq = tc.tile([P, NDC, D], bf16, name="wq_bf")

            def load_w_half(w_bf, W_e, half):
                nc.gpsimd.dma_start(
                    out=w_bf[:, :, half * 512 : (half + 1) * 512],
                    in_=W_e[:, half * 512 : (half + 1) * 512].rearrange(
                        "(o p) n -> p o n", p=P
                    ),
                )

            load_x_chunk(0)
            load_w_half(wk_bf, Wk_e, 0)
            load_x_chunk(1)
            load_w_half(wk_bf, Wk_e, 1)
            load_x_chunk(2)
            load_x_chunk(3)
            nc.gpsimd.dma_start(
                out=wv_bf[:], in_=Wv_e.ap().rearrange("(o p) n -> p o n", p=P)
            )
            load_w_half(wq_bf, Wq_e, 0)

            # E pair-tiles: eTT[s_in, so, (jh, k)] bf16, contiguous 2KB reads
            eTT_tiles = {}

            def stage_eTT(pr):
                eTT = eTT_pool.tile([P, NSC, 2 * LK], bf16, name="eTT")
                nc.gpsimd.dma_start(
                    out=eTT[:],
                    in_=EP_e[pr].rearrange("(o p) n -> p o n", p=P),
                )
                eTT_tiles[pr] = eTT

            stage_eTT(0)
            stage_eTT(1)
            load_w_half(wq_bf, Wq_e, 1)
            stage_eTT(2)

            with (
                tc.tile_pool(name="ps_kvq", bufs=4, space="PSUM") as ps_kvq,
                tc.tile_pool(name="ps_part", bufs=2, space="PSUM") as ps_part,
            ):
                # ---------------- K then V, natural [s, dh] (masked, bf16) ----------
                for t, w_bf in ((0, wk_bf), (1, wv_bf)):
                    for half in range(2):
                        for sc in range(NSC):
                            ps = ps_kvq.tile([P, 512], f32, name="ps_kv", tag="mm512")
                            for dc in range(NDC):
                                nc.tensor.matmul(
                                    ps[:],
                                    xT[:, dc, sc * P : (sc + 1) * P],
                                    w_bf[:, dc, half * 512 : (half + 1) * 512],
                                    start=(dc == 0),
                                    stop=(dc == NDC - 1),
                                )
                            if include_biases:
                                nc.vector.tensor_tensor(
                                    out=ps[:],
                                    in0=ps[:],
                                    in1=bkv_bc[:, t, half * 512 : (half + 1) * 512],
                                    op=mybir.AluOpType.add,
                                )
                            nc.vector.tensor_scalar(
                                out=kv[:, sc, half * 8 : (half + 1) * 8, t, :],
                                in0=ps[:],
                                scalar1=m_sb[:, sc : sc + 1],
                                scalar2=None,
                                op0=mybir.AluOpType.mult,
                            )
                    if t == 0:
                        free_wk()
                free_wv()

                # Allocated only now: reuses the space wk/wv vacated.
                qT_lo, free_qT_lo = tc.tile([P, 4, SL], bf16, name="qT_lo")
                qT_hi, free_qT_hi = tc.tile([P, 4, SL], bf16, name="qT_hi")

                def q_piece(piece):
                    mc, sn = piece // NSN, piece % NSN
                    dst, dj = (qT_lo, 0) if mc < 4 else (qT_hi, 4)
                    ps = ps_kvq.tile([P, 512], f32, name="psq", tag="mm512")
                    for dc in range(NDC):
                        nc.tensor.matmul(
                            ps[:],
                            wq_bf[:, dc, mc * P : (mc + 1) * P],
                            xT[:, dc, sn * 512 : (sn + 1) * 512],
                            start=(dc == 0),
                            stop=(dc == NDC - 1),
                        )
                    nc.vector.tensor_scalar(
                        out=dst[:, mc - dj, sn * 512 : (sn + 1) * 512],
                        in0=ps[:],
                        scalar1=bq_sb[:, mc : mc + 1],
                        scalar2=None,
                        op0=mybir.AluOpType.add,
                    )

                # ---------------- partials per head, Q pieces woven in --------------
                for h in range(H):
                    q_piece(2 * h)
                    q_piece(2 * h + 1)
                    eTT = eTT_tiles[h // 2]
                    jh = h % 2
                    kp_ps = ps_part.tile([P, LK], f32, name="kp_ps")
                    for so in range(NSC):
                        nc.tensor.matmul(
                            kp_ps[:],
                            kv[:, so, h, :, :],
                            eTT[:, so, jh * LK : (jh + 1) * LK],
                            start=(so == 0),
                            stop=(so == NSC - 1),
                        )
                    kp_sb = part_pool.tile([P, LK], bf16, name="kp_sb")
                    nc.vector.tensor_copy(kp_sb[:], kp_ps[:])
                    nc.sync.dma_start(
                        out=cc_in[h].rearrange("(a b) -> a b", a=P),
                        in_=kp_sb[:],
                    )
                    if h == 7:
                        # heads 0-7 AllReduce fires mid-weave, queued ahead of
                        # the next E cast; the h=8..15 weave covers its latency.
                        nc.gpsimd.collective_compute(
                            "AllReduce",
                            mybir.AluOpType.add,
                            replica_groups=PAIRS,
                            ins=[cc_in[0:8].opt()],
                            outs=[cc_out[0:8].opt()],
                        )
                    if h % 2 == 1:
                        pr = h // 2
                        eTT_tiles.pop(pr)
                        if pr + 3 < H // 2:
                            stage_eTT(pr + 3)

                nc.gpsimd.collective_compute(
                    "AllReduce",
                    mybir.AluOpType.add,
                    replica_groups=PAIRS,
                    ins=[cc_in[8:16].opt()],
                    outs=[cc_out[8:16].opt()],
                )

            free_kv()
            eT_scope.close()

            # ---------------- Wo (needed only for the projection tail) ------------
            wo_bf, free_wo = tc.tile([P, NDC, D], bf16, name="wo_bf")
            nc.gpsimd.dma_start(
                out=wo_bf[:], in_=Wo_e.ap().rearrange("(o p) n -> p o n", p=P)
            )
            nc.gpsimd.dma_start(out=bo_bc[:], in_=bo_e.ap()[None, :].to_broadcast((P, D)))

            # ---------------- read back reduced Kp^T / Vp ----------------
            kpT, free_kpT = tc.tile([P, H // 2, LK], bf16, name="kpT")
            vp_sb2, free_vp = tc.tile([P, H, 2, HD], bf16, name="vp_sb2")

            def read_back(h):
                par = (h % 2) * 64
                nc.sync.dma_start(
                    out=kpT[par : par + 64, h // 2, :],
                    in_=cc_out[h, 0 : 64 * LK].rearrange("(a b) -> a b", a=64),
                )
                # Vp^T [hd, k] in DRAM -> XBAR transpose -> vp [k_in, kc, hd]
                nc.sync.dma_start_transpose(
                    vp_sb2[:, h, :, :],
                    cc_out[h, 64 * LK :].rearrange("(a b) -> a b", a=64),
                )

            # heads 0-7 gate only on the first AllReduce; heads 8-15 are read
            # back after two attention pair-groups so the PE covers AR1 latency.
            for h in range(8):
                read_back(h)

            # ---------------- attention (j outer) + output projection tail ---------
            xoT, free_xoT = tc.tile([P, NDC, SL], bf16, name="xoT")
            with (
                tc.tile_pool(name="at_pool", bufs=3) as at_pool,
                tc.tile_pool(name="rbc_pool", bufs=2) as rbc_pool,
                tc.tile_pool(name="osb_pool", bufs=3) as osb_pool,
                tc.tile_pool(name="ps_dot", bufs=3, space="PSUM") as ps_dot,
                tc.tile_pool(name="ps_xoden", bufs=5, space="PSUM") as ps_xoden,
            ):
                if True:
                    def attn_pair(sn, j):
                        # heads (2j, 2j+1): even on partitions 0-63, odd on 64-127
                        ssl = slice(sn * 512, (sn + 1) * 512)
                        ats = []
                        for par in (0, 64):
                            at = at_pool.tile([P, 2, 512], bf16, name="at")
                            for kc in range(2):
                                dps = ps_dot.tile([P, 512], f32, name="dps")
                                qsrc = qT_lo if j < 4 else qT_hi
                                nc.tensor.matmul(
                                    dps[:],
                                    kpT[par : par + 64, j, kc * P : (kc + 1) * P],
                                    qsrc[par : par + 64, j % 4, ssl],
                                    start=True,
                                    stop=True,
                                )
                                nc.scalar.activation(
                                    out=at[:, kc, :],
                                    in_=dps[:],
                                    func=mybir.ActivationFunctionType.Exp,
                                    scale=0.125,
                                )
                            ats.append(at)
                        xo_ps = ps_xoden.tile([P, 512], f32, name="xo_ps", tag="xoden")
                        den_ps = ps_xoden.tile([P, 512], f32, name="den_ps", tag="xoden")
                        for kc in range(2):
                            for pi, par in ((0, 0), (1, 64)):
                                h = 2 * j + pi
                                nc.tensor.matmul(
                                    xo_ps[par : par + 64, :],
                                    vp_sb2[:, h, kc, :],
                                    ats[pi][:, kc, :],
                                    start=(kc == 0),
                                    stop=(kc == 1),
                                    skip_group_check=True,
                                )
                                nc.tensor.matmul(
                                    den_ps[par : par + 64, :],
                                    ones_sb[:],
                                    ats[pi][:, kc, :],
                                    start=(kc == 0),
                                    stop=(kc == 1),
                                    skip_group_check=True,
                                )
                        rbc = rbc_pool.tile([P, 512], f32, name="rbc")
                        nc.vector.reciprocal_approx_fast(out=rbc[:], in_=den_ps[:])
                        nc.vector.tensor_tensor(
                            out=xoT[:, j, ssl],
                            in0=xo_ps[:],
                            in1=rbc[:],
                            op=mybir.AluOpType.mult,
                        )

                    def proj_sn(sn):
                        for si in range(4):
                            sc = sn * 4 + si
                            for half in range(2):
                                ps = ps_xoden.tile([P, 512], f32, name="ps_o", tag="xoden")
                                for c in range(NDC):
                                    nc.tensor.matmul(
                                        ps[:],
                                        xoT[:, c, sc * P : (sc + 1) * P],
                                        wo_bf[:, c, half * 512 : (half + 1) * 512],
                                        start=(c == 0),
                                        stop=(c == NDC - 1),
                                    )
                                osb = osb_pool.tile([P, 512], f32, name="osb")
                                nc.vector.tensor_tensor(
                                    out=osb[:],
                                    in0=ps[:],
                                    in1=bo_bc[:, half * 512 : (half + 1) * 512],
                                    op=mybir.AluOpType.add,
                                )
                                nc.sync.dma_start(
                                    out=out_e[sc * P : (sc + 1) * P, half * 512 : (half + 1) * 512],
                                    in_=osb[:],
                                )

                    # j-major while AR1 is in flight (j<4 only needs AR0)...
                    for j in range(4):
                        for sn in range(NSN):
                            attn_pair(sn, j)
                        if j == 1:
                            # AR1 has had two pair-groups (~20us) of PE cover
                            for h in range(8, H):
                                read_back(h)
                    # ...then sn-major so each sn-group's projection overlaps
                    # the next group's attention.
                    for sn in range(NSN):
                        for j in range(4, H // 2):
                            attn_pair(sn, j)
                        proj_sn(sn)
            free_xoT()
            free_vp()
            free_kpT()
            free_wo()
            free_qT_hi()
            free_qT_lo()
            free_wq()
            free_xT()

    nc.compile()
    return nc


_cache = {}


def _get_nc(include_biases: bool):
    if include_biases not in _cache:
        _cache[include_biases] = _build(include_biases)
    return _cache[include_biases]


def make_in_maps(inputs):
    X = np.asarray(inputs["X"], np.float32)
    mask = np.asarray(inputs["mask"], np.float32)
    E = np.asarray(inputs["E"], np.float32)
    Ws = {k: np.asarray(inputs[k], np.float32) for k in ("Wq", "Wk", "Wv", "Wo")}
    bs = {k: np.asarray(inputs[k], np.float32) for k in ("bq", "bk", "bv", "bo")}

    in_maps = []
    for c in range(8):
        b, half = c // 2, c % 2
        sl = slice(half * SL, (half + 1) * SL)
        # host-side layout prep: X^T and head-pair-packed E^T
        XT = np.ascontiguousarray(X[b, sl, :].T)                    # [D, SL]
        Esl = E[:, :, sl]                                           # [H, LK, SL]
        EP = np.ascontiguousarray(
            Esl.transpose(2, 0, 1).reshape(SL, H // 2, 2 * LK).transpose(1, 0, 2)
        )                                                           # [H/2, SL, 2*LK]
        in_maps.append(
            {
                "XT": XT,
                "mask": np.ascontiguousarray(mask[b, sl]),
                "Wq": Ws["Wq"], "bq": bs["bq"],
                "Wk": Ws["Wk"], "bk": bs["bk"],
                "Wv": Ws["Wv"], "bv": bs["bv"],
                "EP": EP,
                "Wo": Ws["Wo"], "bo": bs["bo"],
            }
        )
    include_biases = bool(np.any(bs["bk"]) or np.any(bs["bv"]))
    return in_maps, include_biases


def kernel(**inputs) -> np.ndarray:
    in_maps, include_biases = make_in_maps(inputs)
    nc = _get_nc(include_biases)
    res = bass_utils.run_bass_kernel_spmd(nc, in_maps, core_ids=list(range(8)))
    out = np.empty((B, S, D), np.float32)
    for c in range(8):
        b, half = c // 2, c % 2
        out[b, half * SL : (half + 1) * SL, :] = res.results[c]["out"]
    return out


# revision 23
# speedup vs baseline: 1.1631x; 1.0463x over previous
"""Trainium2 Bass kernel for Linformer-style sparse attention.

Problem shapes (hardcoded): B=4, S=4096, D=1024, H=16, HD=64, LK=256.

Sharding (8 cores): core c -> (batch b = c//2, sequence half = c%2).
Each core:
  - computes Q/K/V for its 2048 rows (all heads),
  - computes partial [Kp^T; Vp^T] = (K|V)^T @ E^T over its rows,
  - pair AllReduce ([0,1],[2,3],[4,5],[6,7]) completes Kp/Vp (2x 512 KiB bf16),
  - attention (softmax over LK=256) + output projection for its own rows,
  - writes its [2048, 1024] slice of the output directly (no final collective).

Key layout decision: X and E are passed to the device ALREADY TRANSPOSED
(prepared on the host as part of input sharding): XT = X_slice^T [D, SL] and
EP = head-pair-packed E^T [H/2, SL, 2*LK].  On-device they are cast-loaded
f32->bf16 with fully contiguous >=2KB reads straight into matmul-ready SBUF
layouts.  This removes the on-device XBAR transposes of X and E (the previous
bottleneck: tens of thousands of 256-byte transpose packets that saturated
the DMA engines for ~400us and starved the PE).

Other scheduling choices:
  - XT is cast in 4 s-chunks so the first K matmul starts ~20us in.
  - K loop runs before the V loop so only Wk gates compute start.
  - Q^T is computed in 32 (mc, sn) pieces woven two-per-head between partial
    heads; E pairs stream through a 4-deep pool during the K/V phase.
  - The pair AllReduce is split in half (heads 0-7 fire mid-weave, 8-15 at
    the end); attention runs j-outer so the j<4 pair-groups (AR0 results)
    cover AR1's latency; Vp is un-transposed by tiny per-head XBAR DMAs.
  - kp writes and cc readbacks ride the HWDGE (sync) queue; Wo's cast is
    issued after the weave so nothing blocks behind the collectives.
"""

import sys

sys.path.insert(0, "/opt/trn_rl_repo")

from contextlib import ExitStack

import numpy as np

from concourse import bacc, bass_utils, mybir, tile

B, S, D = 4, 4096, 1024
H, HD, LK = 16, 64, 256
SL = S // 2            # local sequence rows per core
P = 128
NSC = SL // P          # 16 s-chunks of 128
NDC = D // P           # 8 d-chunks of 128
NSN = SL // 512        # 4 s-chunks of 512
f32 = mybir.dt.float32
bf16 = mybir.dt.bfloat16
PAIRS = [[0, 1], [2, 3], [4, 5], [6, 7]]


def _build(include_biases: bool, debug: bool = False):
    nc = bacc.Bacc("TRN2", target_bir_lowering=False, num_devices=8)

    XT_e = nc.declare_dram_parameter("XT", [D, SL], f32, isOutput=False)
    mask_e = nc.declare_dram_parameter("mask", [SL], f32, isOutput=False)
    Wq_e = nc.declare_dram_parameter("Wq", [D, D], f32, isOutput=False)
    bq_e = nc.declare_dram_parameter("bq", [D], f32, isOutput=False)
    Wk_e = nc.declare_dram_parameter("Wk", [D, D], f32, isOutput=False)
    bk_e = nc.declare_dram_parameter("bk", [D], f32, isOutput=False)
    Wv_e = nc.declare_dram_parameter("Wv", [D, D], f32, isOutput=False)
    bv_e = nc.declare_dram_parameter("bv", [D], f32, isOutput=False)
    EP_e = nc.declare_dram_parameter("EP", [H // 2, SL, 2 * LK], f32, isOutput=False)
    Wo_e = nc.declare_dram_parameter("Wo", [D, D], f32, isOutput=False)
    bo_e = nc.declare_dram_parameter("bo", [D], f32, isOutput=False)
    out_e = nc.declare_dram_parameter("out", [SL, D], f32, isOutput=True)

    # AllReduce bounce (bf16): per head [KpT ; VpT] stacked [128, 256] flat
    cc_in = nc.dram_tensor("cc_in", [H, P * LK], bf16, kind="Internal")
    cc_out = nc.dram_tensor("cc_out", [H, P * LK], bf16, kind="Internal")

    with tile.TileContext(nc) as tc:
        ctx = ExitStack()
        with ctx:
            const_pool = ctx.enter_context(tc.tile_pool(name="consts", bufs=1))

            # ---------------- constants ----------------
            m_sb = const_pool.tile([P, NSC], f32, name="m_sb")
            nc.sync.dma_start(m_sb[:], mask_e.ap().rearrange("(o p) -> p o", p=P))
            bq_sb = const_pool.tile([P, NDC], f32, name="bq_sb")
            nc.sync.dma_start(bq_sb[:], bq_e.ap().rearrange("(o p) -> p o", p=P))
            if include_biases:
                bkv_bc = const_pool.tile([P, 2, D], f32, name="bkv_bc")
                nc.sync.dma_start(bkv_bc[:, 0, :], bk_e.ap()[None, :].to_broadcast((P, D)))
                nc.sync.dma_start(bkv_bc[:, 1, :], bv_e.ap()[None, :].to_broadcast((P, D)))
            ones_sb = const_pool.tile([P, HD], bf16, name="ones_sb")
            nc.vector.memset(ones_sb[:], 1.0)
            bo_bc = const_pool.tile([P, D], bf16, name="bo_bc")

            # ---------------- left-stack long-lived tiles ----------------
            # xT layout: [d_in(P), dc(8), s(2048)]  (d = dc*128 + d_in)
            xT, free_xT = tc.tile([P, NDC, SL], bf16, name="xT")

            # ---------------- right-stack: freed in reverse order ----------------
            eT_scope = ExitStack()
            eTT_pool = eT_scope.enter_context(
                tc.tile_pool(name="eTT", bufs=3, side="right")
            )
            part_pool = eT_scope.enter_context(
                tc.tile_pool(name="part", bufs=3, side="right")
            )
            kv, free_kv = tc.tile([P, NSC, H, 2, HD], bf16, name="kv", side="right")
            wv_bf, free_wv = tc.tile([P, NDC, D], bf16, name="wv_bf", side="right")
            wk_bf, free_wk = tc.tile([P, NDC, D], bf16, name="wk_bf", side="right")

            # ---------------- cast loads (issue order = SWDGE drain order) --------
            def load_x_chunk(sn):
                nc.gpsimd.dma_start(
                    out=xT[:, :, sn * 512 : (sn + 1) * 512],
                    in_=XT_e[:, sn * 512 : (sn + 1) * 512].rearrange(
                        "(o p) n -> p o n", p=P
                    ),
                )

            wq_bf, free_wq = tc.tile([P, NDC, D], bf16, name="wq_bf")

            def load_w_half(w_bf, W_e, half):
                nc.gpsimd.dma_start(
                    out=w_bf[:, :, half * 512 : (half + 1) * 512],
                    in_=W_e[:, half * 512 : (half + 1) * 512].rearrange(
                        "(o p) n -> p o n", p=P
                    ),
                )

            load_x_chunk(0)
            load_w_half(wk_bf, Wk_e, 0)
            load_x_chunk(1)
            load_w_half(wk_bf, Wk_e, 1)
            load_x_chunk(2)
            load_x_chunk(3)
            nc.gpsimd.dma_start(
                out=wv_bf[:], in_=Wv_e.ap().rearrange("(o p) n -> p o n", p=P)
            )
            load_w_half(wq_bf, Wq_e, 0)

            # E pair-tiles: eTT[s_in, so, (jh, k)] bf16, contiguous 2KB reads
            eTT_tiles = {}

            def stage_eTT(pr):
                eTT = eTT_pool.tile([P, NSC, 2 * LK], bf16, name="eTT")
                nc.gpsimd.dma_start(
                    out=eTT[:],
                    in_=EP_e[pr].rearrange("(o p) n -> p o n", p=P),
                )
                eTT_tiles[pr] = eTT

            stage_eTT(0)
            stage_eTT(1)
            load_w_half(wq_bf, Wq_e, 1)
            stage_eTT(2)

            with (
                tc.tile_pool(name="ps_kvq", bufs=4, space="PSUM") as ps_kvq,
                tc.tile_pool(name="ps_part", bufs=2, space="PSUM") as ps_part,
            ):
                # ---------------- K then V, natural [s, dh] (masked, bf16) ----------
                for t, w_bf in ((0, wk_bf), (1, wv_bf)):
                    for half in range(2):
                        for sc in range(NSC):
                            ps = ps_kvq.tile([P, 512], f32, name="ps_kv", tag="mm512")
                            for dc in range(NDC):
                                nc.tensor.matmul(
                                    ps[:],
                                    xT[:, dc, sc * P : (sc + 1) * P],
                                    w_bf[:, dc, half * 512 : (half + 1) * 512],
                                    start=(dc == 0),
                                    stop=(dc == NDC - 1),
                                )
                            if include_biases:
                                nc.vector.tensor_tensor(
                                    out=ps[:],
                                    in0=ps[:],
                                    in1=bkv_bc[:, t, half * 512 : (half + 1) * 512],
                                    op=mybir.AluOpType.add,
                                )
                            nc.vector.tensor_scalar(
                                out=kv[:, sc, half * 8 : (half + 1) * 8, t, :],
                                in0=ps[:],
                                scalar1=m_sb[:, sc : sc + 1],
                                scalar2=None,
                                op0=mybir.AluOpType.mult,
                            )
                    if t == 0:
                        free_wk()
                free_wv()

                # Allocated only now: reuses the space wk/wv vacated.
                qT_lo, free_qT_lo = tc.tile([P, 4, SL], bf16, name="qT_lo")
                qT_hi, free_qT_hi = tc.tile([P, 4, SL], bf16, name="qT_hi")

                def q_piece(piece):
                    mc, sn = piece // NSN, piece % NSN
                    dst, dj = (qT_lo, 0) if mc < 4 else (qT_hi, 4)
                    ps = ps_kvq.tile([P, 512], f32, name="psq", tag="mm512")
                    for dc in range(NDC):
                        nc.tensor.matmul(
                            ps[:],
                            wq_bf[:, dc, mc * P : (mc + 1) * P],
                            xT[:, dc, sn * 512 : (sn + 1) * 512],
                            start=(dc == 0),
                            stop=(dc == NDC - 1),
                        )
                    nc.vector.tensor_scalar(
                        out=dst[:, mc - dj, sn * 512 : (sn + 1) * 512],
                        in0=ps[:],
                        scalar1=bq_sb[:, mc : mc + 1],
                        scalar2=None,
                        op0=mybir.AluOpType.add,
                    )

                # ---------------- partials per head, Q pieces woven in --------------
                for h in range(H):
                    if h < 8:
                        q_piece(2 * h)
                        q_piece(2 * h + 1)
                    else:
                        q_piece(16 + (h - 8))
                    eTT = eTT_tiles[h // 2]
                    jh = h % 2
                    kp_ps = ps_part.tile([P, LK], f32, name="kp_ps")
                    for so in range(NSC):
                        nc.tensor.matmul(
                            kp_ps[:],
                            kv[:, so, h, :, :],
                            eTT[:, so, jh * LK : (jh + 1) * LK],
                            start=(so == 0),
                            stop=(so == NSC - 1),
                        )
                    kp_sb = part_pool.tile([P, LK], bf16, name="kp_sb")
                    nc.vector.tensor_copy(kp_sb[:], kp_ps[:])
                    nc.sync.dma_start(
                        out=cc_in[h].rearrange("(a b) -> a b", a=P),
                        in_=kp_sb[:],
                    )
                    if h == 7:
                        # heads 0-7 AllReduce fires mid-weave, queued ahead of
                        # the next E cast; the h=8..15 weave covers its latency.
                        nc.gpsimd.collective_compute(
                            "AllReduce",
                            mybir.AluOpType.add,
                            replica_groups=PAIRS,
                            ins=[cc_in[0:8].opt()],
                            outs=[cc_out[0:8].opt()],
                        )
                    if h % 2 == 1:
                        pr = h // 2
                        eTT_tiles.pop(pr)
                        if pr + 3 < H // 2:
                            stage_eTT(pr + 3)

                nc.gpsimd.collective_compute(
                    "AllReduce",
                    mybir.AluOpType.add,
                    replica_groups=PAIRS,
                    ins=[cc_in[8:16].opt()],
                    outs=[cc_out[8:16].opt()],
                )

                # held-back Q pieces (mc 6-7) keep the PE busy while the
                # first AllReduce completes
                for piece in range(24, 32):
                    q_piece(piece)

            free_kv()
            eT_scope.close()

            # ---------------- Wo (needed only for the projection tail) ------------
            wo_bf, free_wo = tc.tile([P, NDC, D], bf16, name="wo_bf")
            nc.gpsimd.dma_start(
                out=wo_bf[:], in_=Wo_e.ap().rearrange("(o p) n -> p o n", p=P)
            )
            nc.gpsimd.dma_start(out=bo_bc[:], in_=bo_e.ap()[None, :].to_broadcast((P, D)))

            # ---------------- read back reduced Kp^T / Vp ----------------
            kpT, free_kpT = tc.tile([P, H // 2, LK], bf16, name="kpT")
            vp_sb2, free_vp = tc.tile([P, H, 2, HD], bf16, name="vp_sb2")

            def read_back(h):
                par = (h % 2) * 64
                nc.sync.dma_start(
                    out=kpT[par : par + 64, h // 2, :],
                    in_=cc_out[h, 0 : 64 * LK].rearrange("(a b) -> a b", a=64),
                )
                # Vp^T [hd, k] in DRAM -> XBAR transpose -> vp [k_in, kc, hd]
                nc.sync.dma_start_transpose(
                    vp_sb2[:, h, :, :],
                    cc_out[h, 64 * LK :].rearrange("(a b) -> a b", a=64),
                )

            # heads 0-7 gate only on the first AllReduce; heads 8-15 are read
            # back after two attention pair-groups so the PE covers AR1 latency.
            for h in range(8):
                read_back(h)

            # ---------------- attention (j outer) + output projection tail ---------
            xoT, free_xoT = tc.tile([P, NDC, SL], bf16, name="xoT")
            with (
                tc.tile_pool(name="at_pool", bufs=3) as at_pool,
                tc.tile_pool(name="rbc_pool", bufs=2) as rbc_pool,
                tc.tile_pool(name="osb_pool", bufs=3) as osb_pool,
                tc.tile_pool(name="ps_dot", bufs=3, space="PSUM") as ps_dot,
                tc.tile_pool(name="ps_xoden", bufs=5, space="PSUM") as ps_xoden,
            ):
                if True:
                    def attn_pair(sn, j):
                        # heads (2j, 2j+1): even on partitions 0-63, odd on 64-127
                        ssl = slice(sn * 512, (sn + 1) * 512)
                        ats = []
                        for par in (0, 64):
                            at = at_pool.tile([P, 2, 512], bf16, name="at")
                            for kc in range(2):
                                dps = ps_dot.tile([P, 512], f32, name="dps")
                                qsrc = qT_lo if j < 4 else qT_hi
                                nc.tensor.matmul(
                                    dps[:],
                                    kpT[par : par + 64, j, kc * P : (kc + 1) * P],
                                    qsrc[par : par + 64, j % 4, ssl],
                                    start=True,
                                    stop=True,
                                )
                                nc.scalar.activation(
                                    out=at[:, kc, :],
                                    in_=dps[:],
                                    func=mybir.ActivationFunctionType.Exp,
                                    scale=0.125,
                                )
                            ats.append(at)
                        xo_ps = ps_xoden.tile([P, 512], f32, name="xo_ps", tag="xoden")
                        den_ps = ps_xoden.tile([P, 512], f32, name="den_ps", tag="xoden")
                        for kc in range(2):
                            for pi, par in ((0, 0), (1, 64)):
                                h = 2 * j + pi
                                nc.tensor.matmul(
                                    xo_ps[par : par + 64, :],
                                    vp_sb2[:, h, kc, :],
                                    ats[pi][:, kc, :],
                                    start=(kc == 0),
                                    stop=(kc == 1),
                                    skip_group_check=True,
                                )
                                nc.tensor.matmul(
                                    den_ps[par : par + 64, :],
                                    ones_sb[:],
                                    ats[pi][:, kc, :],
                                    start=(kc == 0),
                                    stop=(kc == 1),
                                    skip_group_check=True,
                                )
                        rbc = rbc_pool.tile([P, 512], f32, name="rbc")
                        nc.vector.reciprocal_approx_fast(out=rbc[:], in_=den_ps[:])
                        nc.vector.tensor_tensor(
                            out=xoT[:, j, ssl],
                            in0=xo_ps[:],
                            in1=rbc[:],
                            op=mybir.AluOpType.mult,
                        )

                    def proj_sn(sn):
                        for si in range(4):
                            sc = sn * 4 + si
                            for half in range(2):
                                ps = ps_xoden.tile([P, 512], f32, name="ps_o", tag="xoden")
                                for c in range(NDC):
                                    nc.tensor.matmul(
                                        ps[:],
                                        xoT[:, c, sc * P : (sc + 1) * P],
                                        wo_bf[:, c, half * 512 : (half + 1) * 512],
                                        start=(c == 0),
                                        stop=(c == NDC - 1),
                                    )
                                osb = osb_pool.tile([P, 512], f32, name="osb")
                                nc.vector.tensor_tensor(
                                    out=osb[:],
                                    in0=ps[:],
                                    in1=bo_bc[:, half * 512 : (half + 1) * 512],
                                    op=mybir.AluOpType.add,
                                )
                                nc.sync.dma_start(
                                    out=out_e[sc * P : (sc + 1) * P, half * 512 : (half + 1) * 512],
                                    in_=osb[:],
                                )

                    # j-major while AR1 is in flight (j<4 only needs AR0)...
                    for j in range(4):
                        for sn in range(NSN):
                            attn_pair(sn, j)
                        if j == 1:
                            # AR1 has had two pair-groups (~20us) of PE cover;
                            # kpT first: the dot consumes it before vp is needed
                            for h in range(8, H):
                                par = (h % 2) * 64
                                nc.sync.dma_start(
                                    out=kpT[par : par + 64, h // 2, :],
                                    in_=cc_out[h, 0 : 64 * LK].rearrange(
                                        "(a b) -> a b", a=64
                                    ),
                                )
                            for h in range(8, H):
                                nc.sync.dma_start_transpose(
                                    vp_sb2[:, h, :, :],
                                    cc_out[h, 64 * LK :].rearrange("(a b) -> a b", a=64),
                                )
                    # ...then sn-major so each sn-group's projection overlaps
                    # the next group's attention.
                    for sn in range(NSN):
                        for j in range(4, H // 2):
                            attn_pair(sn, j)
                        proj_sn(sn)
            free_xoT()
            free_vp()
            free_kpT()
            free_wo()
            free_qT_hi()
            free_qT_lo()
            free_wq()
            free_xT()

    nc.compile()
    return nc


_cache = {}


def _get_nc(include_biases: bool):
    if include_biases not in _cache:
        _cache[include_biases] = _build(include_biases)
    return _cache[include_biases]


def make_in_maps(inputs):
    X = np.asarray(inputs["X"], np.float32)
    mask = np.asarray(inputs["mask"], np.float32)
    E = np.asarray(inputs["E"], np.float32)
    Ws = {k: np.asarray(inputs[k], np.float32) for k in ("Wq", "Wk", "Wv", "Wo")}
    bs = {k: np.asarray(inputs[k], np.float32) for k in ("bq", "bk", "bv", "bo")}

    in_maps = []
    for c in range(8):
        b, half = c // 2, c % 2
        sl = slice(half * SL, (half + 1) * SL)
        # host-side layout prep: X^T and head-pair-packed E^T
        XT = np.ascontiguousarray(X[b, sl, :].T)                    # [D, SL]
        Esl = E[:, :, sl]                                           # [H, LK, SL]
        EP = np.ascontiguousarray(
            Esl.transpose(2, 0, 1).reshape(SL, H // 2, 2 * LK).transpose(1, 0, 2)
        )                                                           # [H/2, SL, 2*LK]
        in_maps.append(
            {
                "XT": XT,
                "mask": np.ascontiguousarray(mask[b, sl]),
                "Wq": Ws["Wq"], "bq": bs["bq"],
                "Wk": Ws["Wk"], "bk": bs["bk"],
                "Wv": Ws["Wv"], "bv": bs["bv"],
                "EP": EP,
                "Wo": Ws["Wo"], "bo": bs["bo"],
            }
        )
    include_biases = bool(np.any(bs["bk"]) or np.any(bs["bv"]))
    return in_maps, include_biases


def kernel(**inputs) -> np.ndarray:
    in_maps, include_biases = make_in_maps(inputs)
    nc = _get_nc(include_biases)
    res = bass_utils.run_bass_kernel_spmd(nc, in_maps, core_ids=list(range(8)))
    out = np.empty((B, S, D), np.float32)
    for c in range(8):
        b, half = c // 2, c % 2
        out[b, half * SL : (half + 1) * SL, :] = res.results[c]["out"]
    return out


# revision 24
# speedup vs baseline: 1.1659x; 1.0024x over previous
"""Trainium2 Bass kernel for Linformer-style sparse attention.

Problem shapes (hardcoded): B=4, S=4096, D=1024, H=16, HD=64, LK=256.

Sharding (8 cores): core c -> (batch b = c//2, sequence half = c%2).
Each core:
  - computes Q/K/V for its 2048 rows (all heads),
  - computes partial [Kp^T; Vp^T] = (K|V)^T @ E^T over its rows,
  - pair AllReduce ([0,1],[2,3],[4,5],[6,7]) completes Kp/Vp (2x 512 KiB bf16),
  - attention (softmax over LK=256) + output projection for its own rows,
  - writes its [2048, 1024] slice of the output directly (no final collective).

Key layout decision: X and E are passed to the device ALREADY TRANSPOSED
(prepared on the host as part of input sharding): XT = X_slice^T [D, SL] and
EP = head-pair-packed E^T [H/2, SL, 2*LK].  On-device they are cast-loaded
f32->bf16 with fully contiguous >=2KB reads straight into matmul-ready SBUF
layouts.  This removes the on-device XBAR transposes of X and E (the previous
bottleneck: tens of thousands of 256-byte transpose packets that saturated
the DMA engines for ~400us and starved the PE).

Other scheduling choices:
  - XT is cast in 4 s-chunks so the first K matmul starts ~20us in.
  - K loop runs before the V loop so only Wk gates compute start.
  - Q^T is computed in 32 (mc, sn) pieces woven two-per-head between partial
    heads; E pairs stream through a 4-deep pool during the K/V phase.
  - The pair AllReduce is split in half (heads 0-7 fire mid-weave, 8-15 at
    the end); attention runs j-outer so the j<4 pair-groups (AR0 results)
    cover AR1's latency; Vp is un-transposed by tiny per-head XBAR DMAs.
  - kp writes and cc readbacks ride the HWDGE (sync) queue; Wo's cast is
    issued after the weave so nothing blocks behind the collectives.
"""

import sys

sys.path.insert(0, "/opt/trn_rl_repo")

from contextlib import ExitStack

import numpy as np

from concourse import bacc, bass_utils, mybir, tile

B, S, D = 4, 4096, 1024
H, HD, LK = 16, 64, 256
SL = S // 2            # local sequence rows per core
P = 128
NSC = SL // P          # 16 s-chunks of 128
NDC = D // P           # 8 d-chunks of 128
NSN = SL // 512        # 4 s-chunks of 512
f32 = mybir.dt.float32
bf16 = mybir.dt.bfloat16
PAIRS = [[0, 1], [2, 3], [4, 5], [6, 7]]


def _build(include_biases: bool, debug: bool = False):
    nc = bacc.Bacc("TRN2", target_bir_lowering=False, num_devices=8)

    XT_e = nc.declare_dram_parameter("XT", [D, SL], f32, isOutput=False)
    mask_e = nc.declare_dram_parameter("mask", [SL], f32, isOutput=False)
    Wq_e = nc.declare_dram_parameter("Wq", [D, D], f32, isOutput=False)
    bq_e = nc.declare_dram_parameter("bq", [D], f32, isOutput=False)
    Wk_e = nc.declare_dram_parameter("Wk", [D, D], f32, isOutput=False)
    bk_e = nc.declare_dram_parameter("bk", [D], f32, isOutput=False)
    Wv_e = nc.declare_dram_parameter("Wv", [D, D], f32, isOutput=False)
    bv_e = nc.declare_dram_parameter("bv", [D], f32, isOutput=False)
    EP_e = nc.declare_dram_parameter("EP", [H // 2, SL, 2 * LK], f32, isOutput=False)
    Wo_e = nc.declare_dram_parameter("Wo", [D, D], f32, isOutput=False)
    bo_e = nc.declare_dram_parameter("bo", [D], f32, isOutput=False)
    out_e = nc.declare_dram_parameter("out", [SL, D], f32, isOutput=True)

    # AllReduce bounce (bf16): per head [KpT ; VpT] stacked [128, 256] flat
    cc_in = nc.dram_tensor("cc_in", [H, P * LK], bf16, kind="Internal")
    cc_out = nc.dram_tensor("cc_out", [H, P * LK], bf16, kind="Internal")

    with tile.TileContext(nc) as tc:
        ctx = ExitStack()
        with ctx:
            const_pool = ctx.enter_context(tc.tile_pool(name="consts", bufs=1))

            # ---------------- constants ----------------
            m_sb = const_pool.tile([P, NSC], f32, name="m_sb")
            nc.sync.dma_start(m_sb[:], mask_e.ap().rearrange("(o p) -> p o", p=P))
            bq_sb = const_pool.tile([P, NDC], f32, name="bq_sb")
            nc.sync.dma_start(bq_sb[:], bq_e.ap().rearrange("(o p) -> p o", p=P))
            if include_biases:
                bkv_bc = const_pool.tile([P, 2, D], f32, name="bkv_bc")
                nc.sync.dma_start(bkv_bc[:, 0, :], bk_e.ap()[None, :].to_broadcast((P, D)))
                nc.sync.dma_start(bkv_bc[:, 1, :], bv_e.ap()[None, :].to_broadcast((P, D)))
            ones_sb = const_pool.tile([P, HD], bf16, name="ones_sb")
            nc.vector.memset(ones_sb[:], 1.0)
            bo_bc = const_pool.tile([P, D], bf16, name="bo_bc")

            # ---------------- left-stack long-lived tiles ----------------
            # xT layout: [d_in(P), dc(8), s(2048)]  (d = dc*128 + d_in)
            xT, free_xT = tc.tile([P, NDC, SL], bf16, name="xT")

            # ---------------- right-stack: freed in reverse order ----------------
            eT_scope = ExitStack()
            eTT_pool = eT_scope.enter_context(
                tc.tile_pool(name="eTT", bufs=6, side="right")
            )
            part_pool = eT_scope.enter_context(
                tc.tile_pool(name="part", bufs=3, side="right")
            )
            kv, free_kv = tc.tile([P, NSC, H, 2, HD], bf16, name="kv", side="right")
            wv_bf, free_wv = tc.tile([P, NDC, D], bf16, name="wv_bf", side="right")
            wk_bf, free_wk = tc.tile([P, NDC, D], bf16, name="wk_bf", side="right")

            # ---------------- cast loads (issue order = SWDGE drain order) --------
            def load_x_chunk(sn):
                nc.gpsimd.dma_start(
                    out=xT[:, :, sn * 512 : (sn + 1) * 512],
                    in_=XT_e[:, sn * 512 : (sn + 1) * 512].rearrange(
                        "(o p) n -> p o n", p=P
                    ),
                )

            wq_bf, free_wq = tc.tile([P, NDC, D], bf16, name="wq_bf")

            def load_w_half(w_bf, W_e, half):
                nc.gpsimd.dma_start(
                    out=w_bf[:, :, half * 512 : (half + 1) * 512],
                    in_=W_e[:, half * 512 : (half + 1) * 512].rearrange(
                        "(o p) n -> p o n", p=P
                    ),
                )

            def load_x_span(s0, s1):
                nc.gpsimd.dma_start(
                    out=xT[:, :, s0:s1],
                    in_=XT_e[:, s0:s1].rearrange("(o p) n -> p o n", p=P),
                )

            def load_wk_q(dc0, dc1, half):
                nc.gpsimd.dma_start(
                    out=wk_bf[:, dc0:dc1, half * 512 : (half + 1) * 512],
                    in_=Wk_e[dc0 * P : dc1 * P, half * 512 : (half + 1) * 512].rearrange(
                        "(o p) n -> p o n", p=P
                    ),
                )

            load_x_span(0, 256)
            load_wk_q(0, 4, 0)
            load_wk_q(4, 8, 0)
            load_x_span(256, 512)
            load_w_half(wk_bf, Wk_e, 1)
            load_x_chunk(1)
            load_x_chunk(2)
            load_x_chunk(3)
            nc.gpsimd.dma_start(
                out=wv_bf[:], in_=Wv_e.ap().rearrange("(o p) n -> p o n", p=P)
            )
            load_w_half(wq_bf, Wq_e, 0)

            # E head-tiles: eT[s_in, so, k] bf16, 1KB contiguous reads out of
            # the pair-packed EP
            eTT_tiles = {}

            def stage_eTT(h):
                eT = eTT_pool.tile([P, NSC, LK], bf16, name="eT")
                nc.gpsimd.dma_start(
                    out=eT[:],
                    in_=EP_e[h // 2]
                    .rearrange("(o p) (j k) -> p o j k", p=P, j=2)[:, :, h % 2, :],
                )
                eTT_tiles[h] = eT

            stage_eTT(0)
            stage_eTT(1)
            stage_eTT(2)
            load_w_half(wq_bf, Wq_e, 1)
            for h in range(3, 6):
                stage_eTT(h)

            with (
                tc.tile_pool(name="ps_kvq", bufs=4, space="PSUM") as ps_kvq,
                tc.tile_pool(name="ps_part", bufs=2, space="PSUM") as ps_part,
            ):
                # ---------------- K then V, natural [s, dh] (masked, bf16) ----------
                for t, w_bf in ((0, wk_bf), (1, wv_bf)):
                    for half in range(2):
                        for sc in range(NSC):
                            ps = ps_kvq.tile([P, 512], f32, name="ps_kv", tag="mm512")
                            for dc in range(NDC):
                                nc.tensor.matmul(
                                    ps[:],
                                    xT[:, dc, sc * P : (sc + 1) * P],
                                    w_bf[:, dc, half * 512 : (half + 1) * 512],
                                    start=(dc == 0),
                                    stop=(dc == NDC - 1),
                                )
                            if include_biases:
                                nc.vector.tensor_tensor(
                                    out=ps[:],
                                    in0=ps[:],
                                    in1=bkv_bc[:, t, half * 512 : (half + 1) * 512],
                                    op=mybir.AluOpType.add,
                                )
                            nc.vector.tensor_scalar(
                                out=kv[:, sc, half * 8 : (half + 1) * 8, t, :],
                                in0=ps[:],
                                scalar1=m_sb[:, sc : sc + 1],
                                scalar2=None,
                                op0=mybir.AluOpType.mult,
                            )
                    if t == 0:
                        free_wk()
                free_wv()

                # Allocated only now: reuses the space wk/wv vacated.
                qT_lo, free_qT_lo = tc.tile([P, 4, SL], bf16, name="qT_lo")
                qT_hi, free_qT_hi = tc.tile([P, 4, SL], bf16, name="qT_hi")

                def q_piece(piece):
                    mc, sn = piece // NSN, piece % NSN
                    dst, dj = (qT_lo, 0) if mc < 4 else (qT_hi, 4)
                    ps = ps_kvq.tile([P, 512], f32, name="psq", tag="mm512")
                    for dc in range(NDC):
                        nc.tensor.matmul(
                            ps[:],
                            wq_bf[:, dc, mc * P : (mc + 1) * P],
                            xT[:, dc, sn * 512 : (sn + 1) * 512],
                            start=(dc == 0),
                            stop=(dc == NDC - 1),
                        )
                    nc.vector.tensor_scalar(
                        out=dst[:, mc - dj, sn * 512 : (sn + 1) * 512],
                        in0=ps[:],
                        scalar1=bq_sb[:, mc : mc + 1],
                        scalar2=None,
                        op0=mybir.AluOpType.add,
                    )

                # ---------------- partials per head, Q pieces woven in --------------
                for h in range(H):
                    if h < 8:
                        q_piece(2 * h)
                        q_piece(2 * h + 1)
                    else:
                        q_piece(16 + (h - 8))
                    eT = eTT_tiles.pop(h)
                    kp_ps = ps_part.tile([P, LK], f32, name="kp_ps")
                    for so in range(NSC):
                        nc.tensor.matmul(
                            kp_ps[:],
                            kv[:, so, h, :, :],
                            eT[:, so, :],
                            start=(so == 0),
                            stop=(so == NSC - 1),
                        )
                    kp_sb = part_pool.tile([P, LK], bf16, name="kp_sb")
                    nc.vector.tensor_copy(kp_sb[:], kp_ps[:])
                    nc.sync.dma_start(
                        out=cc_in[h].rearrange("(a b) -> a b", a=P),
                        in_=kp_sb[:],
                    )
                    if h == 7:
                        # heads 0-7 AllReduce fires mid-weave, queued ahead of
                        # the next E cast; the h=8..15 weave covers its latency.
                        nc.gpsimd.collective_compute(
                            "AllReduce",
                            mybir.AluOpType.add,
                            replica_groups=PAIRS,
                            ins=[cc_in[0:8].opt()],
                            outs=[cc_out[0:8].opt()],
                        )
                    if h + 6 < H:
                        stage_eTT(h + 6)

                nc.gpsimd.collective_compute(
                    "AllReduce",
                    mybir.AluOpType.add,
                    replica_groups=PAIRS,
                    ins=[cc_in[8:16].opt()],
                    outs=[cc_out[8:16].opt()],
                )

                # held-back Q pieces (mc 6-7) keep the PE busy while the
                # first AllReduce completes
                for piece in range(24, 32):
                    q_piece(piece)

            free_kv()
            eT_scope.close()

            # ---------------- Wo (needed only for the projection tail) ------------
            wo_bf, free_wo = tc.tile([P, NDC, D], bf16, name="wo_bf")
            nc.gpsimd.dma_start(
                out=wo_bf[:], in_=Wo_e.ap().rearrange("(o p) n -> p o n", p=P)
            )
            nc.gpsimd.dma_start(out=bo_bc[:], in_=bo_e.ap()[None, :].to_broadcast((P, D)))

            # ---------------- read back reduced Kp^T / Vp ----------------
            kpT, free_kpT = tc.tile([P, H // 2, LK], bf16, name="kpT")
            vp_sb2, free_vp = tc.tile([P, H, 2, HD], bf16, name="vp_sb2")

            def read_back(h):
                par = (h % 2) * 64
                nc.sync.dma_start(
                    out=kpT[par : par + 64, h // 2, :],
                    in_=cc_out[h, 0 : 64 * LK].rearrange("(a b) -> a b", a=64),
                )
                # Vp^T [hd, k] in DRAM -> XBAR transpose -> vp [k_in, kc, hd]
                nc.sync.dma_start_transpose(
                    vp_sb2[:, h, :, :],
                    cc_out[h, 64 * LK :].rearrange("(a b) -> a b", a=64),
                )

            # heads 0-7 gate only on the first AllReduce; heads 8-15 are read
            # back after two attention pair-groups so the PE covers AR1 latency.
            for h in range(8):
                read_back(h)

            # ---------------- attention (j outer) + output projection tail ---------
            xoT, free_xoT = tc.tile([P, NDC, SL], bf16, name="xoT")
            with (
                tc.tile_pool(name="at_pool", bufs=3) as at_pool,
                tc.tile_pool(name="rbc_pool", bufs=2) as rbc_pool,
                tc.tile_pool(name="osb_pool", bufs=3) as osb_pool,
                tc.tile_pool(name="ps_dot", bufs=3, space="PSUM") as ps_dot,
                tc.tile_pool(name="ps_xoden", bufs=5, space="PSUM") as ps_xoden,
            ):
                if True:
                    def attn_pair(sn, j):
                        # heads (2j, 2j+1): even on partitions 0-63, odd on 64-127
                        ssl = slice(sn * 512, (sn + 1) * 512)
                        ats = []
                        for par in (0, 64):
                            at = at_pool.tile([P, 2, 512], bf16, name="at")
                            for kc in range(2):
                                dps = ps_dot.tile([P, 512], f32, name="dps")
                                qsrc = qT_lo if j < 4 else qT_hi
                                nc.tensor.matmul(
                                    dps[:],
                                    kpT[par : par + 64, j, kc * P : (kc + 1) * P],
                                    qsrc[par : par + 64, j % 4, ssl],
                                    start=True,
                                    stop=True,
                                )
                                nc.scalar.activation(
                                    out=at[:, kc, :],
                                    in_=dps[:],
                                    func=mybir.ActivationFunctionType.Exp,
                                    scale=0.125,
                                )
                            ats.append(at)
                        xo_ps = ps_xoden.tile([P, 512], f32, name="xo_ps", tag="xoden")
                        den_ps = ps_xoden.tile([P, 512], f32, name="den_ps", tag="xoden")
                        for kc in range(2):
                            for pi, par in ((0, 0), (1, 64)):
                                h = 2 * j + pi
                                nc.tensor.matmul(
                                    xo_ps[par : par + 64, :],
                                    vp_sb2[:, h, kc, :],
                                    ats[pi][:, kc, :],
                                    start=(kc == 0),
                                    stop=(kc == 1),
                                    skip_group_check=True,
                                )
                                nc.tensor.matmul(
                                    den_ps[par : par + 64, :],
                                    ones_sb[:],
                                    ats[pi][:, kc, :],
                                    start=(kc == 0),
                                    stop=(kc == 1),
                                    skip_group_check=True,
                                )
                        rbc = rbc_pool.tile([P, 512], f32, name="rbc")
                        nc.vector.reciprocal_approx_fast(out=rbc[:], in_=den_ps[:])
                        nc.vector.tensor_tensor(
                            out=xoT[:, j, ssl],
                            in0=xo_ps[:],
                            in1=rbc[:],
                            op=mybir.AluOpType.mult,
                        )

                    def proj_sn(sn):
                        for si in range(4):
                            sc = sn * 4 + si
                            for half in range(2):
                                ps = ps_xoden.tile([P, 512], f32, name="ps_o", tag="xoden")
                                for c in range(NDC):
                                    nc.tensor.matmul(
                                        ps[:],
                                        xoT[:, c, sc * P : (sc + 1) * P],
                                        wo_bf[:, c, half * 512 : (half + 1) * 512],
                                        start=(c == 0),
                                        stop=(c == NDC - 1),
                                    )
                                osb = osb_pool.tile([P, 512], f32, name="osb")
                                nc.vector.tensor_tensor(
                                    out=osb[:],
                                    in0=ps[:],
                                    in1=bo_bc[:, half * 512 : (half + 1) * 512],
                                    op=mybir.AluOpType.add,
                                )
                                nc.sync.dma_start(
                                    out=out_e[sc * P : (sc + 1) * P, half * 512 : (half + 1) * 512],
                                    in_=osb[:],
                                )

                    # j-major while AR1 is in flight (j<4 only needs AR0)...
                    for j in range(4):
                        for sn in range(NSN):
                            attn_pair(sn, j)
                        if j == 1:
                            # AR1 has had two pair-groups (~20us) of PE cover;
                            # kpT first: the dot consumes it before vp is needed
                            for h in range(8, H):
                                par = (h % 2) * 64
                                nc.sync.dma_start(
                                    out=kpT[par : par + 64, h // 2, :],
                                    in_=cc_out[h, 0 : 64 * LK].rearrange(
                                        "(a b) -> a b", a=64
                                    ),
                                )
                            for h in range(8, H):
                                nc.sync.dma_start_transpose(
                                    vp_sb2[:, h, :, :],
                                    cc_out[h, 64 * LK :].rearrange("(a b) -> a b", a=64),
                                )
                    # ...then sn-major so each sn-group's projection overlaps
                    # the next group's attention.
                    for sn in range(NSN):
                        for j in range(4, H // 2):
                            attn_pair(sn, j)
                        proj_sn(sn)
            free_xoT()
            free_vp()
            free_kpT()
            free_wo()
            free_qT_hi()
            free_qT_lo()
            free_wq()
            free_xT()

    nc.compile()
    return nc


_cache = {}


def _get_nc(include_biases: bool):
    if include_biases not in _cache:
        _cache[include_biases] = _build(include_biases)
    return _cache[include_biases]


def make_in_maps(inputs):
    X = np.asarray(inputs["X"], np.float32)
    mask = np.asarray(inputs["mask"], np.float32)
    E = np.asarray(inputs["E"], np.float32)
    Ws = {k: np.asarray(inputs[k], np.float32) for k in ("Wq", "Wk", "Wv", "Wo")}
    bs = {k: np.asarray(inputs[k], np.float32) for k in ("bq", "bk", "bv", "bo")}

    in_maps = []
    for c in range(8):
        b, half = c // 2, c % 2
        sl = slice(half * SL, (half + 1) * SL)
        # host-side layout prep: X^T and head-pair-packed E^T
        XT = np.ascontiguousarray(X[b, sl, :].T)                    # [D, SL]
        Esl = E[:, :, sl]                                           # [H, LK, SL]
        EP = np.ascontiguousarray(
            Esl.transpose(2, 0, 1).reshape(SL, H // 2, 2 * LK).transpose(1, 0, 2)
        )                                                           # [H/2, SL, 2*LK]
        in_maps.append(
            {
                "XT": XT,
                "mask": np.ascontiguousarray(mask[b, sl]),
                "Wq": Ws["Wq"], "bq": bs["bq"],
                "Wk": Ws["Wk"], "bk": bs["bk"],
                "Wv": Ws["Wv"], "bv": bs["bv"],
                "EP": EP,
                "Wo": Ws["Wo"], "bo": bs["bo"],
            }
        )
    include_biases = bool(np.any(bs["bk"]) or np.any(bs["bv"]))
    return in_maps, include_biases


def kernel(**inputs) -> np.ndarray:
    in_maps, include_biases = make_in_maps(inputs)
    nc = _get_nc(include_biases)
    res = bass_utils.run_bass_kernel_spmd(nc, in_maps, core_ids=list(range(8)))
    out = np.empty((B, S, D), np.float32)
    for c in range(8):
        b, half = c // 2, c % 2
        out[b, half * SL : (half + 1) * SL, :] = res.results[c]["out"]
    return out


# revision 25
# speedup vs baseline: 1.2172x; 1.0440x over previous
"""Trainium2 Bass kernel for Linformer-style sparse attention.

Problem shapes (hardcoded): B=4, S=4096, D=1024, H=16, HD=64, LK=256.

Sharding (8 cores): core c -> (batch b = c//2, sequence half = c%2).
Each core:
  - computes Q/K/V for its 2048 rows (all heads),
  - computes partial [Kp^T; Vp^T] = (K|V)^T @ E^T over its rows,
  - pair AllReduce ([0,1],[2,3],[4,5],[6,7]) completes Kp/Vp (2x 512 KiB bf16),
  - attention (softmax over LK=256) + output projection for its own rows,
  - writes its [2048, 1024] slice of the output directly (no final collective).

Key layout decision: X and E are passed to the device ALREADY TRANSPOSED
(prepared on the host as part of input sharding): XT = X_slice^T [D, SL] and
EP = head-pair-packed E^T [H/2, SL, 2*LK].  On-device they are cast-loaded
f32->bf16 with fully contiguous >=2KB reads straight into matmul-ready SBUF
layouts.  This removes the on-device XBAR transposes of X and E (the previous
bottleneck: tens of thousands of 256-byte transpose packets that saturated
the DMA engines for ~400us and starved the PE).

Other scheduling choices:
  - XT is cast in 4 s-chunks so the first K matmul starts ~20us in.
  - K loop runs before the V loop so only Wk gates compute start.
  - Q^T is computed in 32 (mc, sn) pieces woven two-per-head between partial
    heads; E pairs stream through a 4-deep pool during the K/V phase.
  - The pair AllReduce is split in half (heads 0-7 fire mid-weave, 8-15 at
    the end); attention runs j-outer so the j<4 pair-groups (AR0 results)
    cover AR1's latency; Vp is un-transposed by tiny per-head XBAR DMAs.
  - kp writes and cc readbacks ride the HWDGE (sync) queue; Wo's cast is
    issued after the weave so nothing blocks behind the collectives.
"""

import sys

sys.path.insert(0, "/opt/trn_rl_repo")

from contextlib import ExitStack

import numpy as np

from concourse import bacc, bass_utils, mybir, tile

B, S, D = 4, 4096, 1024
H, HD, LK = 16, 64, 256
SL = S // 2            # local sequence rows per core
P = 128
NSC = SL // P          # 16 s-chunks of 128
NDC = D // P           # 8 d-chunks of 128
NSN = SL // 512        # 4 s-chunks of 512
f32 = mybir.dt.float32
bf16 = mybir.dt.bfloat16
PAIRS = [[0, 1], [2, 3], [4, 5], [6, 7]]


def _build(include_biases: bool, debug: bool = False):
    nc = bacc.Bacc("TRN2", target_bir_lowering=False, num_devices=8)

    XT_e = nc.declare_dram_parameter("XT", [D, SL], f32, isOutput=False)
    mask_e = nc.declare_dram_parameter("mask", [SL], f32, isOutput=False)
    Wq_e = nc.declare_dram_parameter("Wq", [D, D], f32, isOutput=False)
    bq_e = nc.declare_dram_parameter("bq", [D], f32, isOutput=False)
    Wk_e = nc.declare_dram_parameter("Wk", [D, D], f32, isOutput=False)
    bk_e = nc.declare_dram_parameter("bk", [D], f32, isOutput=False)
    Wv_e = nc.declare_dram_parameter("Wv", [D, D], f32, isOutput=False)
    bv_e = nc.declare_dram_parameter("bv", [D], f32, isOutput=False)
    EP_e = nc.declare_dram_parameter("EP", [H // 2, SL, 2 * LK], f32, isOutput=False)
    Wo_e = nc.declare_dram_parameter("Wo", [D, D], f32, isOutput=False)
    bo_e = nc.declare_dram_parameter("bo", [D], f32, isOutput=False)
    out_e = nc.declare_dram_parameter("out", [SL, D], f32, isOutput=True)

    # AllReduce bounce (bf16): per head [KpT ; VpT] stacked [128, 256] flat
    cc_in = nc.dram_tensor("cc_in", [H, P * LK], bf16, kind="Internal")
    cc_out = nc.dram_tensor("cc_out", [H, P * LK], bf16, kind="Internal")

    with tile.TileContext(nc) as tc:
        ctx = ExitStack()
        with ctx:
            const_pool = ctx.enter_context(tc.tile_pool(name="consts", bufs=1))

            # ---------------- constants ----------------
            m_sb = const_pool.tile([P, NSC], f32, name="m_sb")
            nc.sync.dma_start(m_sb[:], mask_e.ap().rearrange("(o p) -> p o", p=P))
            bq_sb = const_pool.tile([P, NDC], f32, name="bq_sb")
            nc.sync.dma_start(bq_sb[:], bq_e.ap().rearrange("(o p) -> p o", p=P))
            if include_biases:
                bkv_bc = const_pool.tile([P, 2, D], f32, name="bkv_bc")
                nc.sync.dma_start(bkv_bc[:, 0, :], bk_e.ap()[None, :].to_broadcast((P, D)))
                nc.sync.dma_start(bkv_bc[:, 1, :], bv_e.ap()[None, :].to_broadcast((P, D)))
            ones_sb = const_pool.tile([P, HD], bf16, name="ones_sb")
            nc.vector.memset(ones_sb[:], 1.0)
            bo_bc = const_pool.tile([P, D], bf16, name="bo_bc")

            # ---------------- left-stack long-lived tiles ----------------
            # xT layout: [d_in(P), dc(8), s(2048)]  (d = dc*128 + d_in)
            xT, free_xT = tc.tile([P, NDC, SL], bf16, name="xT")

            # ---------------- right-stack: freed in reverse order ----------------
            eT_scope = ExitStack()
            eTT_pool = eT_scope.enter_context(
                tc.tile_pool(name="eTT", bufs=6, side="right")
            )
            part_pool = eT_scope.enter_context(
                tc.tile_pool(name="part", bufs=3, side="right")
            )
            kv, free_kv = tc.tile([P, NSC, H, 2, HD], bf16, name="kv", side="right")
            wv_bf, free_wv = tc.tile([P, NDC, D], bf16, name="wv_bf", side="right")
            wk_bf, free_wk = tc.tile([P, NDC, D], bf16, name="wk_bf", side="right")

            # ---------------- cast loads (issue order = SWDGE drain order) --------
            def load_x_chunk(sn):
                nc.gpsimd.dma_start(
                    out=xT[:, :, sn * 512 : (sn + 1) * 512],
                    in_=XT_e[:, sn * 512 : (sn + 1) * 512].rearrange(
                        "(o p) n -> p o n", p=P
                    ),
                )

            wq_bf, free_wq = tc.tile([P, NDC, D], bf16, name="wq_bf")

            def load_w_half(w_bf, W_e, half):
                nc.gpsimd.dma_start(
                    out=w_bf[:, :, half * 512 : (half + 1) * 512],
                    in_=W_e[:, half * 512 : (half + 1) * 512].rearrange(
                        "(o p) n -> p o n", p=P
                    ),
                )

            def load_x_span(s0, s1):
                nc.gpsimd.dma_start(
                    out=xT[:, :, s0:s1],
                    in_=XT_e[:, s0:s1].rearrange("(o p) n -> p o n", p=P),
                )

            def load_wk_q(dc0, dc1, half):
                nc.gpsimd.dma_start(
                    out=wk_bf[:, dc0:dc1, half * 512 : (half + 1) * 512],
                    in_=Wk_e[dc0 * P : dc1 * P, half * 512 : (half + 1) * 512].rearrange(
                        "(o p) n -> p o n", p=P
                    ),
                )

            load_x_span(0, 256)
            load_wk_q(0, 4, 0)
            load_wk_q(4, 8, 0)
            load_x_span(256, 512)
            load_w_half(wk_bf, Wk_e, 1)
            load_x_chunk(1)
            load_x_chunk(2)
            load_x_chunk(3)
            nc.gpsimd.dma_start(
                out=wv_bf[:], in_=Wv_e.ap().rearrange("(o p) n -> p o n", p=P)
            )
            load_w_half(wq_bf, Wq_e, 0)

            # E head-tiles: eT[s_in, so, k] bf16, 1KB contiguous reads out of
            # the pair-packed EP
            eTT_tiles = {}

            def stage_eTT(h):
                eT = eTT_pool.tile([P, NSC, LK], bf16, name="eT")
                nc.gpsimd.dma_start(
                    out=eT[:],
                    in_=EP_e[h // 2]
                    .rearrange("(o p) (j k) -> p o j k", p=P, j=2)[:, :, h % 2, :],
                )
                eTT_tiles[h] = eT

            stage_eTT(0)
            stage_eTT(1)
            stage_eTT(2)
            load_w_half(wq_bf, Wq_e, 1)
            for h in range(3, 6):
                stage_eTT(h)

            with (
                tc.tile_pool(name="ps_kvq", bufs=4, space="PSUM") as ps_kvq,
                tc.tile_pool(name="ps_part", bufs=2, space="PSUM") as ps_part,
            ):
                # ---------------- K / V halves, natural [s, dh] (masked, bf16) -----
                def kv_half(t, w_bf, half):
                    for sc in range(NSC):
                        ps = ps_kvq.tile([P, 512], f32, name="ps_kv", tag="mm512")
                        for dc in range(NDC):
                            nc.tensor.matmul(
                                ps[:],
                                xT[:, dc, sc * P : (sc + 1) * P],
                                w_bf[:, dc, half * 512 : (half + 1) * 512],
                                start=(dc == 0),
                                stop=(dc == NDC - 1),
                            )
                        if include_biases:
                            nc.vector.tensor_tensor(
                                out=ps[:],
                                in0=ps[:],
                                in1=bkv_bc[:, t, half * 512 : (half + 1) * 512],
                                op=mybir.AluOpType.add,
                            )
                        nc.vector.tensor_scalar(
                            out=kv[:, sc, half * 8 : (half + 1) * 8, t, :],
                            in0=ps[:],
                            scalar1=m_sb[:, sc : sc + 1],
                            scalar2=None,
                            op0=mybir.AluOpType.mult,
                        )

                def partial_head(h):
                    eT = eTT_tiles.pop(h)
                    kp_ps = ps_part.tile([P, LK], f32, name="kp_ps")
                    for so in range(NSC):
                        nc.tensor.matmul(
                            kp_ps[:],
                            kv[:, so, h, :, :],
                            eT[:, so, :],
                            start=(so == 0),
                            stop=(so == NSC - 1),
                        )
                    kp_sb = part_pool.tile([P, LK], bf16, name="kp_sb")
                    nc.vector.tensor_copy(kp_sb[:], kp_ps[:])
                    nc.sync.dma_start(
                        out=cc_in[h].rearrange("(a b) -> a b", a=P),
                        in_=kp_sb[:],
                    )
                    if h + 6 < H:
                        stage_eTT(h + 6)

                kv_half(0, wk_bf, 0)
                kv_half(0, wk_bf, 1)
                free_wk()
                kv_half(1, wv_bf, 0)

                # partials for heads 0-7 right after V-half0 -> AR0 fires ~50us
                # earlier; V-half1 + all of Q then cover both AllReduces.
                for h in range(8):
                    partial_head(h)
                nc.gpsimd.collective_compute(
                    "AllReduce",
                    mybir.AluOpType.add,
                    replica_groups=PAIRS,
                    ins=[cc_in[0:8].opt()],
                    outs=[cc_out[0:8].opt()],
                )

                kv_half(1, wv_bf, 1)
                free_wv()

                # Allocated only now: reuses the space wk/wv vacated.
                qT_lo, free_qT_lo = tc.tile([P, 4, SL], bf16, name="qT_lo")
                qT_hi, free_qT_hi = tc.tile([P, 4, SL], bf16, name="qT_hi")

                for h in range(8, H):
                    partial_head(h)
                nc.gpsimd.collective_compute(
                    "AllReduce",
                    mybir.AluOpType.add,
                    replica_groups=PAIRS,
                    ins=[cc_in[8:16].opt()],
                    outs=[cc_out[8:16].opt()],
                )

                def q_piece(piece):
                    mc, sn = piece // NSN, piece % NSN
                    dst, dj = (qT_lo, 0) if mc < 4 else (qT_hi, 4)
                    ps = ps_kvq.tile([P, 512], f32, name="psq", tag="mm512")
                    for dc in range(NDC):
                        nc.tensor.matmul(
                            ps[:],
                            wq_bf[:, dc, mc * P : (mc + 1) * P],
                            xT[:, dc, sn * 512 : (sn + 1) * 512],
                            start=(dc == 0),
                            stop=(dc == NDC - 1),
                        )
                    nc.vector.tensor_scalar(
                        out=dst[:, mc - dj, sn * 512 : (sn + 1) * 512],
                        in0=ps[:],
                        scalar1=bq_sb[:, mc : mc + 1],
                        scalar2=None,
                        op0=mybir.AluOpType.add,
                    )

                # ---------------- all Q pieces: AllReduce latency cover -------------
                for piece in range(32):
                    q_piece(piece)

            free_kv()
            eT_scope.close()

            # ---------------- Wo (needed only for the projection tail) ------------
            wo_bf, free_wo = tc.tile([P, NDC, D], bf16, name="wo_bf")
            nc.gpsimd.dma_start(
                out=wo_bf[:], in_=Wo_e.ap().rearrange("(o p) n -> p o n", p=P)
            )
            nc.gpsimd.dma_start(out=bo_bc[:], in_=bo_e.ap()[None, :].to_broadcast((P, D)))

            # ---------------- read back reduced Kp^T / Vp ----------------
            kpT, free_kpT = tc.tile([P, H // 2, LK], bf16, name="kpT")
            vp_sb2, free_vp = tc.tile([P, H, 2, HD], bf16, name="vp_sb2")

            def read_back(h):
                par = (h % 2) * 64
                nc.sync.dma_start(
                    out=kpT[par : par + 64, h // 2, :],
                    in_=cc_out[h, 0 : 64 * LK].rearrange("(a b) -> a b", a=64),
                )
                # Vp^T [hd, k] in DRAM -> XBAR transpose -> vp [k_in, kc, hd]
                nc.sync.dma_start_transpose(
                    vp_sb2[:, h, :, :],
                    cc_out[h, 64 * LK :].rearrange("(a b) -> a b", a=64),
                )

            # heads 0-7 gate only on the first AllReduce; heads 8-15 are read
            # back after two attention pair-groups so the PE covers AR1 latency.
            for h in range(8):
                read_back(h)

            # ---------------- attention (j outer) + output projection tail ---------
            xoT, free_xoT = tc.tile([P, NDC, SL], bf16, name="xoT")
            with (
                tc.tile_pool(name="at_pool", bufs=3) as at_pool,
                tc.tile_pool(name="rbc_pool", bufs=2) as rbc_pool,
                tc.tile_pool(name="osb_pool", bufs=3) as osb_pool,
                tc.tile_pool(name="ps_dot", bufs=3, space="PSUM") as ps_dot,
                tc.tile_pool(name="ps_xoden", bufs=5, space="PSUM") as ps_xoden,
            ):
                if True:
                    def attn_pair(sn, j):
                        # heads (2j, 2j+1): even on partitions 0-63, odd on 64-127
                        ssl = slice(sn * 512, (sn + 1) * 512)
                        ats = []
                        for par in (0, 64):
                            at = at_pool.tile([P, 2, 512], bf16, name="at")
                            for kc in range(2):
                                dps = ps_dot.tile([P, 512], f32, name="dps")
                                qsrc = qT_lo if j < 4 else qT_hi
                                nc.tensor.matmul(
                                    dps[:],
                                    kpT[par : par + 64, j, kc * P : (kc + 1) * P],
                                    qsrc[par : par + 64, j % 4, ssl],
                                    start=True,
                                    stop=True,
                                )
                                nc.scalar.activation(
                                    out=at[:, kc, :],
                                    in_=dps[:],
                                    func=mybir.ActivationFunctionType.Exp,
                                    scale=0.125,
                                )
                            ats.append(at)
                        xo_ps = ps_xoden.tile([P, 512], f32, name="xo_ps", tag="xoden")
                        den_ps = ps_xoden.tile([P, 512], f32, name="den_ps", tag="xoden")
                        for kc in range(2):
                            for pi, par in ((0, 0), (1, 64)):
                                h = 2 * j + pi
                                nc.tensor.matmul(
                                    xo_ps[par : par + 64, :],
                                    vp_sb2[:, h, kc, :],
                                    ats[pi][:, kc, :],
                                    start=(kc == 0),
                                    stop=(kc == 1),
                                    skip_group_check=True,
                                )
                                nc.tensor.matmul(
                                    den_ps[par : par + 64, :],
                                    ones_sb[:],
                                    ats[pi][:, kc, :],
                                    start=(kc == 0),
                                    stop=(kc == 1),
                                    skip_group_check=True,
                                )
                        rbc = rbc_pool.tile([P, 512], f32, name="rbc")
                        nc.vector.reciprocal_approx_fast(out=rbc[:], in_=den_ps[:])
                        nc.vector.tensor_tensor(
                            out=xoT[:, j, ssl],
                            in0=xo_ps[:],
                            in1=rbc[:],
                            op=mybir.AluOpType.mult,
                        )

                    def proj_sn(sn):
                        for si in range(4):
                            sc = sn * 4 + si
                            for half in range(2):
                                ps = ps_xoden.tile([P, 512], f32, name="ps_o", tag="xoden")
                                for c in range(NDC):
                                    nc.tensor.matmul(
                                        ps[:],
                                        xoT[:, c, sc * P : (sc + 1) * P],
                                        wo_bf[:, c, half * 512 : (half + 1) * 512],
                                        start=(c == 0),
                                        stop=(c == NDC - 1),
                                    )
                                osb = osb_pool.tile([P, 512], f32, name="osb")
                                nc.vector.tensor_tensor(
                                    out=osb[:],
                                    in0=ps[:],
                                    in1=bo_bc[:, half * 512 : (half + 1) * 512],
                                    op=mybir.AluOpType.add,
                                )
                                nc.sync.dma_start(
                                    out=out_e[sc * P : (sc + 1) * P, half * 512 : (half + 1) * 512],
                                    in_=osb[:],
                                )

                    # j-major while AR1 is in flight (j<4 only needs AR0)...
                    for j in range(4):
                        for sn in range(NSN):
                            attn_pair(sn, j)
                        if j == 1:
                            # AR1 has had two pair-groups (~20us) of PE cover;
                            # kpT first: the dot consumes it before vp is needed
                            for h in range(8, H):
                                par = (h % 2) * 64
                                nc.sync.dma_start(
                                    out=kpT[par : par + 64, h // 2, :],
                                    in_=cc_out[h, 0 : 64 * LK].rearrange(
                                        "(a b) -> a b", a=64
                                    ),
                                )
                            for h in range(8, H):
                                nc.sync.dma_start_transpose(
                                    vp_sb2[:, h, :, :],
                                    cc_out[h, 64 * LK :].rearrange("(a b) -> a b", a=64),
                                )
                    # ...then sn-major so each sn-group's projection overlaps
                    # the next group's attention.
                    for sn in range(NSN):
                        for j in range(4, H // 2):
                            attn_pair(sn, j)
                        proj_sn(sn)
            free_xoT()
            free_vp()
            free_kpT()
            free_wo()
            free_qT_hi()
            free_qT_lo()
            free_wq()
            free_xT()

    nc.compile()
    return nc


_cache = {}


def _get_nc(include_biases: bool):
    if include_biases not in _cache:
        _cache[include_biases] = _build(include_biases)
    return _cache[include_biases]


def make_in_maps(inputs):
    X = np.asarray(inputs["X"], np.float32)
    mask = np.asarray(inputs["mask"], np.float32)
    E = np.asarray(inputs["E"], np.float32)
    Ws = {k: np.asarray(inputs[k], np.float32) for k in ("Wq", "Wk", "Wv", "Wo")}
    bs = {k: np.asarray(inputs[k], np.float32) for k in ("bq", "bk", "bv", "bo")}

    in_maps = []
    for c in range(8):
        b, half = c // 2, c % 2
        sl = slice(half * SL, (half + 1) * SL)
        # host-side layout prep: X^T and head-pair-packed E^T
        XT = np.ascontiguousarray(X[b, sl, :].T)                    # [D, SL]
        Esl = E[:, :, sl]                                           # [H, LK, SL]
        EP = np.ascontiguousarray(
            Esl.transpose(2, 0, 1).reshape(SL, H // 2, 2 * LK).transpose(1, 0, 2)
        )                                                           # [H/2, SL, 2*LK]
        in_maps.append(
            {
                "XT": XT,
                "mask": np.ascontiguousarray(mask[b, sl]),
                "Wq": Ws["Wq"], "bq": bs["bq"],
                "Wk": Ws["Wk"], "bk": bs["bk"],
                "Wv": Ws["Wv"], "bv": bs["bv"],
                "EP": EP,
                "Wo": Ws["Wo"], "bo": bs["bo"],
            }
        )
    include_biases = bool(np.any(bs["bk"]) or np.any(bs["bv"]))
    return in_maps, include_biases


def kernel(**inputs) -> np.ndarray:
    in_maps, include_biases = make_in_maps(inputs)
    nc = _get_nc(include_biases)
    res = bass_utils.run_bass_kernel_spmd(nc, in_maps, core_ids=list(range(8)))
    out = np.empty((B, S, D), np.float32)
    for c in range(8):
        b, half = c // 2, c % 2
        out[b, half * SL : (half + 1) * SL, :] = res.results[c]["out"]
    return out
